# revision 1
# baseline (speedup 1.0000x reference)
"""Trainium2 Bass kernel for nn_Attention_3710851743764.

Full attention block: qkv proj -> per-head RMSNorm(q,k) -> RoPE -> GQA
attention (16 q heads, 4 kv heads, S=2048, D=128) -> out proj.

Sharding: 8 cores = 2 (batch) x 4 (kv-head groups). Each core computes its
batch's qkv for its group (4 q heads + 1 kv head), full attention for those
heads, and a partial output projection (its 512 wo columns); the host sums
the 4 partials per batch.

Dataflow is fully "transposed" (features on partitions, tokens on free):
  qkvT[f,t]   = mm(lhsT=wqkvT[d,f], rhs=xT[d,t])            accumulated over d
  ssq[c,t]    = mm(lhsT=esel[:,c,:], rhs=square(qkvT_c))     (RMS factors)
  rot[d',t]   = mm(lhsT=P_rot, rhs=qn)                       (RoPE pair swap)
  scoresT[s,t]= mm(lhsT=kT[:,s-blk], rhs=qT_h)               per 128-s block
  pT          = exp(scoresT)          (no max subtraction: |score|<=sqrt(128))
  attnT[d,t]  = mm(lhsT=v[s-blk,d], rhs=pT)                  accumulated over s
  denom[h,t]  = mm(lhsT=esel[:,h,:4], rhs=pT)                accumulated
  out[t,o]    = mm(lhsT=attnT_n[f,t-blk], rhs=woT[f,o])      accumulated over f

All matmuls run in float32r (tf32-like, full PE rate).
"""

import sys

sys.path.insert(0, "/opt/trn_rl_repo")

import numpy as np

import concourse.bass as bass
import concourse.tile as tile
from concourse import bacc, mybir
from concourse import bass_utils

F32 = mybir.dt.float32
F32R = mybir.dt.float32r
AF = mybir.ActivationFunctionType
OP = mybir.AluOpType

DIM = 2048
N_HEADS = 16
N_KV = 4
HEAD_DIM = 128
B = 2
S = 2048
EPS = float(np.finfo(np.float32).eps)
GQ = N_HEADS // N_KV          # q heads per group = 4
GF = GQ * HEAD_DIM            # group q features = 512
P = 128
KC = DIM // P                 # 16 contraction chunks for projections
TC = 4                        # token chunks of 512
SC = S // P                   # 16 key chunks of 128
NF = GF + 2 * HEAD_DIM        # 768 qkv features per group
FC = NF // P                  # 6 feature chunks

_CACHED_NC = None


def build_nc():
    """Build the single-core Bass program (same program for all 8 cores)."""
    nc = bacc.Bacc("TRN2", target_bir_lowering=False, debug=False,
                   num_devices=8)

    xT_d = nc.dram_tensor("xT", [TC, P, KC, 512], F32R,
                          kind="ExternalInput").ap()
    wqkvT_d = nc.dram_tensor("wqkvT", [P, KC, NF], F32R,
                             kind="ExternalInput").ap()
    woT_d = nc.dram_tensor("woT", [HEAD_DIM, GQ, DIM], F32R,
                           kind="ExternalInput").ap()
    cosT_d = nc.dram_tensor("cosT", [HEAD_DIM, S], F32,
                            kind="ExternalInput").ap()
    sinT_d = nc.dram_tensor("sinT", [HEAD_DIM, S], F32,
                            kind="ExternalInput").ap()
    normw_d = nc.dram_tensor("normw", [P, 2], F32, kind="ExternalInput").ap()
    prot_d = nc.dram_tensor("prot", [P, P], F32R, kind="ExternalInput").ap()
    ident_d = nc.dram_tensor("ident", [P, P], F32R, kind="ExternalInput").ap()
    esel_d = nc.dram_tensor("esel", [P, 5, 5], F32R,
                            kind="ExternalInput").ap()
    out_d = nc.dram_tensor("out", [SC, P, TC, 512], F32,
                           kind="ExternalOutput").ap()

    with tile.TileContext(nc) as tc:
        with (
            tc.tile_pool(name="consts", bufs=1) as cp,
        ):
            dramp = tc.alloc_tile_pool(name="dram_scratch", bufs=1,
                                       space="DRAM")
            rfac_dr = dramp.tile([5, S], F32, name="rfac_dr")
            rd_dr = [dramp.tile([4, 1024], F32, name=f"rd_dr{i}")
                     for i in range(2)]
            # stage-scoped persistent tensors (manual release for SBUF reuse)
            p1 = tc.alloc_tile_pool(name="p1", bufs=1)   # lives A..C
            qkv_raw = p1.tile([P, 5, S], F32, name="qkv_raw")     # 40KB
            vT_sb = p1.tile([P, S], F32R, name="vT_sb")           # 8KB
            rfac = p1.tile([5, S], F32, name="rfac")

            cos_sb = cp.tile([HEAD_DIM, S], F32, name="cos_sb")
            sin_sb = cp.tile([HEAD_DIM, S], F32, name="sin_sb")
            normw_sb = cp.tile([P, 2], F32, name="normw_sb")
            prot_sb = cp.tile([P, P], F32R, name="prot_sb")
            ident_sb = cp.tile([P, P], F32R, name="ident_sb")
            esel_sb = cp.tile([P, 5, 5], F32R, name="esel_sb")
            eps_sb = cp.tile([P, 1], F32, name="eps_sb")
            zero_sb = cp.tile([P, 1], F32, name="zero_sb")
            nc.vector.memset(eps_sb[:], EPS)
            nc.vector.memset(zero_sb[:], 0.0)
            nc.sync.dma_start(esel_sb[:], esel_d)

            # ---------------- Stage A: qkv projection + squares ----------
            # fc-outer so each weight chunk (lhsT) is reused across the
            # 4 token chunks (amortizes LDWEIGHTS 4x); full xT resident.
            with (
                tc.tile_pool(name="stA", bufs=2) as sa,
                tc.tile_pool(name="wq_pool", bufs=1) as wp,
                tc.tile_pool(name="psA", bufs=4, space="PSUM") as psA,
                tc.tile_pool(name="psSq", bufs=1, space="PSUM") as psSq,
            ):
                wq_sb = wp.tile([P, KC, NF], F32R, name="wq_sb")  # 48KB
                ssq_ps = psSq.tile([5, S], F32, name="ssq_ps")    # 4 banks
                for tcc in range(TC):
                    xt = sa.tile([P, KC, 512], F32R, name="xt")   # 32KB x2
                    for kc4 in range(0, KC, 4):
                        nc.sync.dma_start(xt[:, kc4:kc4 + 4, :],
                                          xT_d[tcc, :, kc4:kc4 + 4, :])
                        if tcc == 0:
                            nc.sync.dma_start(wq_sb[:, kc4:kc4 + 4, :],
                                              wqkvT_d[:, kc4:kc4 + 4, :])
                    for fc in range(FC):
                        ps = psA.tile([P, 512], F32, name="qkv_ps")
                        for kc in range(KC):
                            nc.tensor.matmul(
                                ps[:],
                                wq_sb[:, kc, fc * P:(fc + 1) * P],
                                xt[:, kc, :],
                                start=(kc == 0), stop=(kc == KC - 1))
                        tsl = slice(tcc * 512, (tcc + 1) * 512)
                        if fc < 5:
                            sq = sa.tile([P, 512], F32R, name="sq")
                            nc.scalar.activation(sq[:], ps[:], AF.Square,
                                                 bias=zero_sb[:])
                            nc.tensor.matmul(
                                ssq_ps[:, tsl], esel_sb[:, fc, :],
                                sq[:],
                                start=(fc == 0), stop=(fc == 4),
                                skip_group_check=True)
                            nc.vector.tensor_copy(qkv_raw[:, fc, tsl], ps[:])
                        else:
                            nc.vector.tensor_copy(vT_sb[:, tsl], ps[:])
                nc.sync.dma_start(cos_sb[:], cosT_d)
                nc.sync.dma_start(sin_sb[:], sinT_d)
                nc.sync.dma_start(normw_sb[:], normw_d)
                nc.sync.dma_start(prot_sb[:], prot_d)
                nc.sync.dma_start(ident_sb[:], ident_d)
                # Stage B: rms factors  rfac = 1/sqrt(ssq/128 + eps)
                std = wp.tile([5, S], F32, name="std")
                nc.scalar.activation(std[:], ssq_ps[:], AF.Sqrt,
                                     scale=1.0 / HEAD_DIM, bias=eps_sb[0:5, :])
                nc.vector.reciprocal_approx_fast(rfac[:], std[:])
                nc.gpsimd.dma_start(rfac_dr[:], rfac[:])

            # ---------------- Stage C: normalize + rope + v transpose ----
            p2 = tc.alloc_tile_pool(name="p2", bufs=1, side="right")  # C..D
            qk_sb = [p2.tile([P, S], F32R, name=f"qk_sb{i}")
                     for i in range(5)]                           # 40KB
            v_sb = p2.tile([P, SC, HEAD_DIM], F32R, name="v_sb")  # 8KB
            # scores psum allocated BEFORE stage C so its banks are not
            # WAR-blocked on C's rope/transpose psum (lets stage D start
            # while C's DVE tail is still running)
            psS = tc.alloc_tile_pool(name="psS", bufs=4, space="PSUM")
            with (
                tc.tile_pool(name="stC", bufs=2) as sc_pool,
                tc.tile_pool(name="psC", bufs=2, space="PSUM") as psC,
            ):
                for fc in (4, 0, 1, 2, 3):
                    rb = sc_pool.tile([P, S], F32, name="rb")
                    nc.gpsimd.dma_start(
                        rb[:], rfac_dr[fc:fc + 1, :].to_broadcast((P, S)))
                    qn = sc_pool.tile([P, S], F32R, name="qn")
                    wcol = 0 if fc < 4 else 1
                    nc.vector.scalar_tensor_tensor(
                        qn[:], qkv_raw[:, fc, :],
                        normw_sb[:, wcol:wcol + 1], rb[:],
                        op0=OP.mult, op1=OP.mult)
                    for tcc in range(TC):
                        tsl = slice(tcc * 512, (tcc + 1) * 512)
                        rot_ps = psC.tile([P, 512], F32, name="rot_ps")
                        nc.tensor.matmul(rot_ps[:], prot_sb[:], qn[:, tsl],
                                         start=True, stop=True)
                        rs = sc_pool.tile([P, 512], F32, name="rs")
                        nc.vector.tensor_mul(rs[:], rot_ps[:], sin_sb[:, tsl])
                        qc = sc_pool.tile([P, 512], F32, name="qc")
                        nc.gpsimd.tensor_mul(qc[:], qn[:, tsl].bitcast(F32),
                                             cos_sb[:, tsl])
                        nc.vector.tensor_add(qk_sb[fc][:, tsl], qc[:],
                                             rs[:])
                for scc in range(SC):
                    vt_ps = psC.tile([P, P], F32R, name="vt_ps")
                    nc.tensor.transpose(
                        vt_ps[:], vT_sb[:, scc * P:(scc + 1) * P],
                        ident_sb[:])
                    nc.vector.tensor_copy(v_sb[:, scc, :], vt_ps[:])

            p1.release()

            # ---------------- Stage D: attention ------------------------
            p3 = tc.alloc_tile_pool(name="p3", bufs=1)   # lives D..E
            atn_raw = [p3.tile([P, GQ, 1024], F32, name=f"atn_raw{i}")
                       for i in range(2)]                         # 32KB
            atn_n = [p3.tile([P, GQ, 1024], F32R, name=f"atn_n{i}")
                     for i in range(2)]                           # 32KB
            woT_sb = p3.tile([P, GQ, DIM], F32R, name="woT_sb")   # 32KB
            with (
                tc.tile_pool(name="stD", bufs=2) as sd,
                tc.tile_pool(name="ptp", bufs=6) as ptp,
                tc.tile_pool(name="psPV", bufs=1, space="PSUM") as psPV,
                tc.tile_pool(name="psDN", bufs=1, space="PSUM") as psDN,
            ):
                for pair in range(2):
                    po = pair * 1024
                    dn_ps = psDN.tile([4, 1024], F32, name="dn_ps")
                    for h in range(GQ):
                        pv_ps = psPV.tile([P, 1024], F32, name="pv_ps")
                        for scc in range(SC):
                            ksl = qk_sb[4][:, scc * P:(scc + 1) * P]
                            for half in range(2):
                                hs = slice(half * 512, half * 512 + 512)
                                sp = psS.tile([P, 512], F32, name="sp")
                                nc.tensor.matmul(
                                    sp[:], ksl,
                                    qk_sb[h][:, po + half * 512:
                                              po + half * 512 + 512],
                                    start=True, stop=True)
                                pt = ptp.tile([P, 512], F32R, name="pt")
                                nc.scalar.activation(pt[:], sp[:], AF.Exp,
                                                     bias=zero_sb[:])
                                nc.tensor.matmul(
                                    pv_ps[:, hs], v_sb[:, scc, :], pt[:],
                                    start=(scc == 0), stop=(scc == SC - 1))
                                nc.tensor.matmul(
                                    dn_ps[:, hs], esel_sb[:, h, 0:4],
                                    pt[:],
                                    start=(h == 0 and scc == 0),
                                    stop=(h == GQ - 1 and scc == SC - 1),
                                    skip_group_check=True)
                        nc.vector.tensor_copy(atn_raw[pair][:, h, :],
                                              pv_ps[:])
                    rd = sd.tile([4, 1024], F32, name="rd")
                    nc.vector.reciprocal_approx_fast(rd[:], dn_ps[:])
                    nc.gpsimd.dma_start(rd_dr[pair][:], rd[:])
                    for h in range(GQ):
                        rbh = sd.tile([P, 1024], F32, name="rbh")
                        nc.gpsimd.dma_start(
                            rbh[:],
                            rd_dr[pair][h:h + 1, :].to_broadcast((P, 1024)))
                        nc.vector.tensor_mul(atn_n[pair][:, h, :],
                                             atn_raw[pair][:, h, :],
                                             rbh[:])

            psS.release()
            p2.release()

            # ---------------- Stage E: output projection -----------------
            nc.sync.dma_start(woT_sb[:], woT_d)
            with (
                tc.tile_pool(name="stE", bufs=4) as se,
                tc.tile_pool(name="psE", bufs=8, space="PSUM") as psE,
            ):
                for tcc in range(SC):
                    pr = tcc // 8
                    tloc = (tcc % 8) * P
                    ps4 = [psE.tile([P, 512], F32, name="out_ps")
                           for _ in range(TC)]
                    for h in range(GQ):
                        lhs = atn_n[pr][:, h, tloc:tloc + P]
                        for oc in range(TC):
                            nc.tensor.matmul(
                                ps4[oc][:], lhs,
                                woT_sb[:, h, oc * 512:(oc + 1) * 512],
                                start=(h == 0), stop=(h == GQ - 1),
                                skip_group_check=True)
                    ob = se.tile([P, TC, 512], F32, name="ob")
                    for oc in range(TC):
                        nc.vector.tensor_copy(ob[:, oc, :], ps4[oc][:])
                    nc.sync.dma_start(out_d[tcc], ob[:])
            p3.release()

    nc.compile()
    return nc


def make_in_maps(x, wqkv, wo, q_norm_w, k_norm_w, freqs_cos, freqs_sin):
    """Build the 8 per-core input maps. Core c = b*4 + g."""
    x = np.asarray(x, np.float32)
    wqkv = np.asarray(wqkv, np.float32)
    wo = np.asarray(wo, np.float32)
    q_norm_w = np.asarray(q_norm_w, np.float32)
    k_norm_w = np.asarray(k_norm_w, np.float32)
    cosT = np.ascontiguousarray(
        np.asarray(freqs_cos, np.float32)[:, 0, :].T)
    sinT = np.ascontiguousarray(
        np.asarray(freqs_sin, np.float32)[:, 0, :].T)

    normw = np.empty((P, 2), np.float32)
    normw[:, 0] = q_norm_w * np.float32(1.0 / np.sqrt(HEAD_DIM))
    normw[:, 1] = k_norm_w

    prot = np.zeros((P, P), np.float32)
    prot[np.arange(1, P, 2), np.arange(0, P, 2)] = -1.0
    prot[np.arange(0, P, 2), np.arange(1, P, 2)] = 1.0
    ident = np.eye(P, dtype=np.float32)
    esel = np.zeros((P, 5, 5), np.float32)
    for c in range(5):
        esel[:, c, c] = 1.0

    q_size = N_HEADS * HEAD_DIM
    kv_size = N_KV * HEAD_DIM
    in_maps = []
    for b in range(B):
        # [tc, p, kc, u]: xT[kc*128+p, tc*512+u] pre-tiled for 1-run/partition
        xT = np.ascontiguousarray(
            x[b].reshape(TC, 512, KC, P).transpose(0, 3, 2, 1))
        for g in range(N_KV):
            wq = wqkv[g * GF:(g + 1) * GF]
            wk = wqkv[q_size + g * HEAD_DIM:q_size + (g + 1) * HEAD_DIM]
            wv = wqkv[q_size + kv_size + g * HEAD_DIM:
                      q_size + kv_size + (g + 1) * HEAD_DIM]
            wqkvT = np.ascontiguousarray(
                np.concatenate([wq, wk, wv], axis=0).T
                .reshape(KC, P, NF).transpose(1, 0, 2))
            woT = np.ascontiguousarray(
                wo[:, g * GF:(g + 1) * GF].T.reshape(GQ, HEAD_DIM, DIM)
                .transpose(1, 0, 2))
            in_maps.append({
                "xT": xT, "wqkvT": wqkvT, "woT": woT,
                "cosT": cosT, "sinT": sinT, "normw": normw,
                "prot": prot, "ident": ident, "esel": esel,
            })
    return in_maps


def run(in_maps, trace=False):
    global _CACHED_NC
    if _CACHED_NC is None:
        _CACHED_NC = build_nc()
    return bass_utils.run_bass_kernel_spmd(
        _CACHED_NC, in_maps, core_ids=list(range(8)), trace=trace)


def kernel(x, wqkv, wo, q_norm_w, k_norm_w, freqs_cos, freqs_sin):
    in_maps = make_in_maps(x, wqkv, wo, q_norm_w, k_norm_w,
                           freqs_cos, freqs_sin)
    res = run(in_maps, trace=False)
    out = np.zeros((B, S, DIM), np.float32)
    for b in range(B):
        for g in range(N_KV):
            o = res.results[b * N_KV + g]["out"]    # [SC, P, TC, 512]
            out[b] += o.transpose(0, 1, 2, 3).reshape(S, DIM)
    return out



# revision 7
# speedup vs baseline: 1.1975x; 1.1975x over previous
"""Trainium2 Bass kernel for nn_Attention_3710851743764.

Full attention block: qkv proj -> per-head RMSNorm(q,k) -> RoPE -> GQA
attention (16 q heads, 4 kv heads, S=2048, D=128) -> out proj.

Sharding: 8 cores = 2 (batch) x 4 (kv-head groups). Each core computes its
batch's qkv for its group (4 q heads + 1 kv head), full attention for those
heads, and a partial output projection (its 512 wo columns); the host sums
the 4 partials per batch.

v2 vs baseline: all matmul inputs in fp16 (FWL weight loads, 2x DVE rate,
half DMA), RMSNorm/RoPE fused into the projection stage per token chunk
(no PE idle gap -> HAM stays at max p-state), 1024-wide exp tiles (halves
ACT per-instruction overhead), stage-D loop kept PE-bound.

Dataflow is fully "transposed" (features on partitions, tokens on free):
  qkvT[f,t]   = mm(lhsT=wqkvT[d,f], rhs=xT[d,t])            accumulated over d
  ssq[c,t]    = mm(lhsT=esel[:,c,:], rhs=square(qkvT_c))     (RMS factors)
  rot[d',t]   = mm(lhsT=P_rot, rhs=qn)                       (RoPE pair swap)
  scoresT[s,t]= mm(lhsT=kT[:,s-blk], rhs=qT_h)               per 128-s block
  pT          = exp(scoresT)          (no max subtraction: |score|<=sqrt(128))
  attnT[d,t]  = mm(lhsT=v[s-blk,d], rhs=pT)                  accumulated over s
  denom[h,t]  = mm(lhsT=esel[:,h,:4], rhs=pT)                accumulated
  out[t,o]    = mm(lhsT=attnT_n[f,t-blk], rhs=woT[f,o])      accumulated over f
"""

import sys

sys.path.insert(0, "/opt/trn_rl_repo")

import numpy as np

import concourse.bass as bass
import concourse.tile as tile
from concourse import bacc, mybir
from concourse import bass_utils

F32 = mybir.dt.float32
F16 = mybir.dt.float16
AF = mybir.ActivationFunctionType
OP = mybir.AluOpType

DIM = 2048
N_HEADS = 16
N_KV = 4
HEAD_DIM = 128
B = 2
S = 2048
EPS = float(np.finfo(np.float32).eps)
GQ = N_HEADS // N_KV          # q heads per group = 4
GF = GQ * HEAD_DIM            # group q features = 512
P = 128
KC = DIM // P                 # 16 contraction chunks for projections
TC = 2                        # token chunks of 1024
SC = S // P                   # 16 key chunks of 128
NF = GF + 2 * HEAD_DIM        # 768 qkv features per group
FC = NF // P                  # 6 feature chunks

_CACHED_NC = None


def build_nc():
    """Build the single-core Bass program (same program for all 8 cores)."""
    nc = bacc.Bacc("TRN2", target_bir_lowering=False, debug=False,
                   num_devices=8)

    xT_d = nc.dram_tensor("xT", [TC, P, KC, 1024], F16,
                          kind="ExternalInput").ap()
    wqkvT_d = nc.dram_tensor("wqkvT", [P, KC, NF], F16,
                             kind="ExternalInput").ap()
    woT_d = nc.dram_tensor("woT", [HEAD_DIM, GQ, DIM], F16,
                           kind="ExternalInput").ap()
    cosT_d = nc.dram_tensor("cosT", [HEAD_DIM, S], F16,
                            kind="ExternalInput").ap()
    sinT_d = nc.dram_tensor("sinT", [HEAD_DIM, S], F16,
                            kind="ExternalInput").ap()
    normw_d = nc.dram_tensor("normw", [P, 2], F32, kind="ExternalInput").ap()
    prot_d = nc.dram_tensor("prot", [P, P], F16, kind="ExternalInput").ap()
    ident_d = nc.dram_tensor("ident", [P, P], F16, kind="ExternalInput").ap()
    esel_d = nc.dram_tensor("esel", [P, 5, 5], F16,
                            kind="ExternalInput").ap()
    out_d = nc.dram_tensor("out", [SC, P, GQ, 512], F16,
                           kind="ExternalOutput").ap()

    with tile.TileContext(nc) as tc:
        with (
            tc.tile_pool(name="consts", bufs=1) as cp,
        ):
            dramp = tc.alloc_tile_pool(name="dram_scratch", bufs=1,
                                       space="DRAM")
            rfac_dr = dramp.tile([5, S], F16, name="rfac_dr")
            rd_dr = [dramp.tile([4, 1024], F16, name=f"rd_dr{i}")
                     for i in range(2)]
            # stage-scoped persistent tensors (manual release for SBUF reuse)
            p1 = tc.alloc_tile_pool(name="p1", bufs=1)   # lives A (raw qkv)
            qkv_raw = p1.tile([P, 5, S], F16, name="qkv_raw")     # 20KB
            vT_sb = p1.tile([P, S], F16, name="vT_sb")            # 4KB

            p2 = tc.alloc_tile_pool(name="p2", bufs=1, side="right")  # A..D
            qk_sb = [p2.tile([P, S], F16, name=f"qk_sb{i}")
                     for i in range(5)]                           # 20KB
            v_sb = p2.tile([P, SC, HEAD_DIM], F16, name="v_sb")   # 4KB

            cos_sb = cp.tile([HEAD_DIM, S], F16, name="cos_sb")
            sin_sb = cp.tile([HEAD_DIM, S], F16, name="sin_sb")
            normw_sb = cp.tile([P, 2], F32, name="normw_sb")
            prot_sb = cp.tile([P, P], F16, name="prot_sb")
            ident_sb = cp.tile([P, P], F16, name="ident_sb")
            esel_sb = cp.tile([P, 5, 5], F16, name="esel_sb")
            eps_sb = cp.tile([P, 1], F32, name="eps_sb")
            zero_sb = cp.tile([P, 1], F32, name="zero_sb")
            nc.vector.memset(eps_sb[:], EPS)
            nc.vector.memset(zero_sb[:], 0.0)
            nc.sync.dma_start(esel_sb[:], esel_d)
            nc.sync.dma_start(normw_sb[:], normw_d)
            nc.sync.dma_start(prot_sb[:], prot_d)
            nc.sync.dma_start(ident_sb[:], ident_d)
            nc.sync.dma_start(cos_sb[:], cosT_d)
            nc.sync.dma_start(sin_sb[:], sinT_d)

            # ---------------- Stage A: qkv proj + rmsnorm + rope + vT ----
            # Per 1024-token chunk: project (PE), squares (ACT) + esel-matmul
            # row sums (PE) -> rms factors -> DRAM-broadcast -> rope
            # (DVE/gpsimd + PE rotate matmul). Rope for chunk t is emitted
            # after projection of chunk t+1 so the PE never waits on the
            # rfac DMA roundtrip.
            with (
                tc.tile_pool(name="stA", bufs=2) as sa,
                tc.tile_pool(name="stR", bufs=3) as sr,
                tc.tile_pool(name="wq_pool", bufs=1) as wp,
                tc.tile_pool(name="psA", bufs=2, space="PSUM") as psA,
                tc.tile_pool(name="psSq", bufs=1, space="PSUM") as psSq,
                tc.tile_pool(name="psRT", bufs=2, space="PSUM") as psRT,
            ):
                wq_sb = wp.tile([P, KC, NF], F16, name="wq_sb")   # 24KB
                rfac16 = [wp.tile([5, 1024], F16, name=f"rfac16_{t}")
                          for t in range(TC)]

                def emit_rope(tcc):
                    """RMSNorm + RoPE for token chunk tcc (k row first)."""
                    tbase = tcc * 1024
                    for fc in (4, 0, 1, 2, 3):
                        rb = sr.tile([P, 1024], F16, name="rb")
                        nc.gpsimd.dma_start(
                            rb[:], rfac_dr[fc:fc + 1, tbase:tbase + 1024]
                            .to_broadcast((P, 1024)))
                        qn = sr.tile([P, 1024], F16, name="qn")
                        wcol = 0 if fc < 4 else 1
                        nc.vector.scalar_tensor_tensor(
                            qn[:], qkv_raw[:, fc, tbase:tbase + 1024],
                            normw_sb[:, wcol:wcol + 1], rb[:],
                            op0=OP.mult, op1=OP.mult)
                        qc = sr.tile([P, 1024], F16, name="qc")
                        nc.gpsimd.tensor_mul(qc[:], qn[:],
                                             cos_sb[:, tbase:tbase + 1024])
                        for hf in range(2):
                            tsl = slice(tbase + hf * 512,
                                        tbase + hf * 512 + 512)
                            lsl = slice(hf * 512, hf * 512 + 512)
                            rot_ps = psRT.tile([P, 512], F32, name="rot_ps",
                                               tag="rt")
                            nc.tensor.matmul(rot_ps[:], prot_sb[:],
                                             qn[:, lsl],
                                             start=True, stop=True)
                            rs = sr.tile([P, 512], F16, name="rs")
                            nc.vector.tensor_mul(rs[:], rot_ps[:],
                                                 sin_sb[:, tsl])
                            nc.vector.tensor_add(qk_sb[fc][:, tsl],
                                                 qc[:, lsl], rs[:])
                    # v transpose for this chunk's 8 key blocks
                    for scc in range(tcc * 8, tcc * 8 + 8):
                        vt_ps = psRT.tile([P, P], F16, name="vt_ps",
                                          tag="rt")
                        nc.tensor.transpose(
                            vt_ps[:], vT_sb[:, scc * P:(scc + 1) * P],
                            ident_sb[:])
                        nc.vector.tensor_copy(v_sb[:, scc, :], vt_ps[:])

                for tcc in range(TC):
                    tbase = tcc * 1024
                    xt = sa.tile([P, KC, 1024], F16, name="xt")   # 32KB x2
                    for kc4 in range(0, KC, 4):
                        nc.sync.dma_start(xt[:, kc4:kc4 + 4, :],
                                          xT_d[tcc, :, kc4:kc4 + 4, :])
                        if tcc == 0:
                            nc.sync.dma_start(wq_sb[:, kc4:kc4 + 4, :],
                                              wqkvT_d[:, kc4:kc4 + 4, :])
                    ssq_ps = psSq.tile([5, 1024], F32, name="ssq_ps")
                    for fc in range(FC):
                        ps = psA.tile([P, 1024], F32, name="qkv_ps")
                        for kc in range(KC):
                            for hf in range(2):
                                hsl = slice(hf * 512, hf * 512 + 512)
                                nc.tensor.matmul(
                                    ps[:, hsl],
                                    wq_sb[:, kc, fc * P:(fc + 1) * P],
                                    xt[:, kc, hsl],
                                    start=(kc == 0), stop=(kc == KC - 1))
                        if fc < 5:
                            sq = sa.tile([P, 1024], F16, name="sq")
                            nc.scalar.activation(sq[:], ps[:], AF.Square,
                                                 bias=zero_sb[:])
                            for hf in range(2):
                                hsl = slice(hf * 512, hf * 512 + 512)
                                nc.tensor.matmul(
                                    ssq_ps[:, hsl], esel_sb[:, fc, :],
                                    sq[:, hsl],
                                    start=(fc == 0), stop=(fc == 4),
                                    skip_group_check=True)
                            nc.vector.tensor_copy(
                                qkv_raw[:, fc, tbase:tbase + 1024], ps[:])
                        else:
                            nc.vector.tensor_copy(
                                vT_sb[:, tbase:tbase + 1024], ps[:])
                    # rms factors for this chunk:
                    # rfac = 1/sqrt(ssq/128 + eps), via DRAM for broadcast
                    std = sa.tile([5, 1024], F32, name="std")
                    nc.scalar.activation(std[:], ssq_ps[:], AF.Sqrt,
                                         scale=1.0 / HEAD_DIM,
                                         bias=eps_sb[0:5, :])
                    rfacf = sa.tile([5, 1024], F32, name="rfacf")
                    nc.vector.reciprocal_approx_fast(rfacf[:], std[:])
                    nc.vector.tensor_copy(rfac16[tcc][:], rfacf[:])
                    nc.gpsimd.dma_start(rfac_dr[:, tbase:tbase + 1024],
                                        rfac16[tcc][:])
                    if tcc > 0:
                        emit_rope(tcc - 1)
                emit_rope(TC - 1)

            p1.release()

            # ---------------- Stage D: attention ------------------------
            p3 = tc.alloc_tile_pool(name="p3", bufs=1)   # lives D..E
            atn_n = [p3.tile([P, GQ, 1024], F16, name=f"atn_n{i}")
                     for i in range(2)]                           # 16KB
            woT_sb = p3.tile([P, GQ, DIM], F16, name="woT_sb")    # 16KB
            nc.sync.dma_start(woT_sb[:], woT_d)
            with (
                tc.tile_pool(name="stD", bufs=2) as sd,
                tc.tile_pool(name="atr", bufs=4) as atr,
                tc.tile_pool(name="ptp", bufs=4) as ptp,
                tc.tile_pool(name="psS", bufs=2, space="PSUM") as psS,
                tc.tile_pool(name="psPV", bufs=1, space="PSUM") as psPV,
                tc.tile_pool(name="psDN", bufs=1, space="PSUM") as psDN,
            ):
                def emit_pv_dn(pv_ps, dn_ps, h, pt, scc):
                    """PV accumulation + denominator rows for one tile."""
                    for hf in range(2):
                        hs = slice(hf * 512, hf * 512 + 512)
                        nc.tensor.matmul(
                            pv_ps[:, hs], v_sb[:, scc, :], pt[:, hs],
                            start=(scc == 0), stop=(scc == SC - 1))
                        nc.tensor.matmul(
                            dn_ps[:, hs], esel_sb[:, h, 0:4], pt[:, hs],
                            start=(h == 0 and scc == 0),
                            stop=(h == GQ - 1 and scc == SC - 1),
                            skip_group_check=True)

                atrs = []
                for pair in range(2):
                    po = pair * 1024
                    dn_ps = psDN.tile([4, 1024], F32, name="dn_ps")
                    for h in range(GQ):
                        pv_ps = psPV.tile([P, 1024], F32, name="pv_ps")
                        prev = None
                        for scc in range(SC):
                            ksl = qk_sb[4][:, scc * P:(scc + 1) * P]
                            sp = psS.tile([P, 1024], F32, name="sp")
                            for hf in range(2):
                                hs = slice(hf * 512, hf * 512 + 512)
                                nc.tensor.matmul(
                                    sp[:, hs], ksl,
                                    qk_sb[h][:, po + hf * 512:
                                              po + hf * 512 + 512],
                                    start=True, stop=True)
                            pt = ptp.tile([P, 1024], F16, name="pt")
                            nc.scalar.activation(pt[:], sp[:], AF.Exp,
                                                 bias=zero_sb[:])
                            if prev is not None:
                                emit_pv_dn(pv_ps, dn_ps, h, *prev)
                            prev = (pt, scc)
                        emit_pv_dn(pv_ps, dn_ps, h, *prev)
                        atn_raw = atr.tile([P, 1024], F16, name="atn_raw")
                        nc.vector.tensor_copy(atn_raw[:], pv_ps[:])
                        atrs.append((pair, h, atn_raw))
                    # softmax denominators -> reciprocal -> broadcast
                    rdf = sd.tile([4, 1024], F32, name="rdf")
                    nc.vector.reciprocal_approx_fast(rdf[:], dn_ps[:])
                    rd16 = sd.tile([4, 1024], F16, name="rd16")
                    nc.vector.tensor_copy(rd16[:], rdf[:])
                    nc.gpsimd.dma_start(rd_dr[pair][:], rd16[:])
                    for h in range(GQ):
                        rbh = sd.tile([P, 1024], F16, name="rbh")
                        nc.gpsimd.dma_start(
                            rbh[:],
                            rd_dr[pair][h:h + 1, :].to_broadcast((P, 1024)))
                        pr, hh, araw = atrs.pop(0)
                        nc.vector.tensor_mul(atn_n[pr][:, hh, :],
                                             araw[:], rbh[:])

            p2.release()

            # ---------------- Stage E: output projection -----------------
            with (
                tc.tile_pool(name="stE", bufs=4) as se,
                tc.tile_pool(name="psE", bufs=8, space="PSUM") as psE,
            ):
                for tcc in range(SC):
                    pr = tcc // 8
                    tloc = (tcc % 8) * P
                    ps4 = [psE.tile([P, 512], F32, name="out_ps")
                           for _ in range(4)]
                    for h in range(GQ):
                        lhs = atn_n[pr][:, h, tloc:tloc + P]
                        for oc in range(4):
                            nc.tensor.matmul(
                                ps4[oc][:], lhs,
                                woT_sb[:, h, oc * 512:(oc + 1) * 512],
                                start=(h == 0), stop=(h == GQ - 1),
                                skip_group_check=True)
                    ob = se.tile([P, GQ, 512], F16, name="ob")
                    for oc in range(4):
                        if oc % 2 == 0:
                            nc.vector.tensor_copy(ob[:, oc, :], ps4[oc][:])
                        else:
                            nc.scalar.activation(ob[:, oc, :], ps4[oc][:],
                                                 AF.Copy)
                    nc.sync.dma_start(out_d[tcc], ob[:])
            p3.release()

    nc.compile()
    return nc


def make_in_maps(x, wqkv, wo, q_norm_w, k_norm_w, freqs_cos, freqs_sin):
    """Build the 8 per-core input maps. Core c = b*4 + g."""
    x = np.asarray(x, np.float32)
    wqkv = np.asarray(wqkv, np.float32)
    wo = np.asarray(wo, np.float32)
    q_norm_w = np.asarray(q_norm_w, np.float32)
    k_norm_w = np.asarray(k_norm_w, np.float32)
    cosT = np.ascontiguousarray(
        np.asarray(freqs_cos, np.float32)[:, 0, :].T).astype(np.float16)
    sinT = np.ascontiguousarray(
        np.asarray(freqs_sin, np.float32)[:, 0, :].T).astype(np.float16)

    normw = np.empty((P, 2), np.float32)
    normw[:, 0] = q_norm_w * np.float32(1.0 / np.sqrt(HEAD_DIM))
    normw[:, 1] = k_norm_w

    prot = np.zeros((P, P), np.float16)
    prot[np.arange(1, P, 2), np.arange(0, P, 2)] = -1.0
    prot[np.arange(0, P, 2), np.arange(1, P, 2)] = 1.0
    ident = np.eye(P, dtype=np.float16)
    esel = np.zeros((P, 5, 5), np.float16)
    for c in range(5):
        esel[:, c, c] = 1.0

    q_size = N_HEADS * HEAD_DIM
    kv_size = N_KV * HEAD_DIM
    in_maps = []
    for b in range(B):
        # [tc, p, kc, u]: xT[kc*128+p, tc*1024+u] pre-tiled, 2KB runs
        xT = np.ascontiguousarray(
            x[b].reshape(TC, 1024, KC, P).transpose(0, 3, 2, 1)
        ).astype(np.float16)
        for g in range(N_KV):
            wq = wqkv[g * GF:(g + 1) * GF]
            wk = wqkv[q_size + g * HEAD_DIM:q_size + (g + 1) * HEAD_DIM]
            wv = wqkv[q_size + kv_size + g * HEAD_DIM:
                      q_size + kv_size + (g + 1) * HEAD_DIM]
            wqkvT = np.ascontiguousarray(
                np.concatenate([wq, wk, wv], axis=0).T
                .reshape(KC, P, NF).transpose(1, 0, 2)).astype(np.float16)
            woT = np.ascontiguousarray(
                wo[:, g * GF:(g + 1) * GF].T.reshape(GQ, HEAD_DIM, DIM)
                .transpose(1, 0, 2)).astype(np.float16)
            in_maps.append({
                "xT": xT, "wqkvT": wqkvT, "woT": woT,
                "cosT": cosT, "sinT": sinT, "normw": normw,
                "prot": prot, "ident": ident, "esel": esel,
            })
    return in_maps


def run(in_maps, trace=False):
    global _CACHED_NC
    if _CACHED_NC is None:
        _CACHED_NC = build_nc()
    return bass_utils.run_bass_kernel_spmd(
        _CACHED_NC, in_maps, core_ids=list(range(8)), trace=trace)


def kernel(x, wqkv, wo, q_norm_w, k_norm_w, freqs_cos, freqs_sin):
    in_maps = make_in_maps(x, wqkv, wo, q_norm_w, k_norm_w,
                           freqs_cos, freqs_sin)
    res = run(in_maps, trace=False)
    out = np.zeros((B, S, DIM), np.float32)
    for b in range(B):
        for g in range(N_KV):
            o = res.results[b * N_KV + g]["out"]    # [SC, P, GQ, 512]
            out[b] += o.reshape(S, DIM).astype(np.float32)
    return out


# revision 12
# speedup vs baseline: 1.4946x; 1.2481x over previous
"""Trainium2 Bass kernel for nn_Attention_3710851743764.

Full attention block: qkv proj -> per-head RMSNorm(q,k) -> RoPE -> GQA
attention (16 q heads, 4 kv heads, S=2048, D=128) -> out proj.

Sharding: 8 cores = 2 (batch) x 4 (kv-head groups). Each core computes its
batch's qkv for its group (4 q heads + 1 kv head), full attention for those
heads, and a partial output projection (its 512 wo columns); the host sums
the 4 partials per batch.

v2 vs baseline: all matmul inputs in fp16 (FWL weight loads, 2x DVE rate,
half DMA), RMSNorm/RoPE fused into the projection stage per token chunk
(no PE idle gap -> HAM stays at max p-state), 1024-wide exp tiles (halves
ACT per-instruction overhead), stage-D loop kept PE-bound.

Dataflow is fully "transposed" (features on partitions, tokens on free):
  qkvT[f,t]   = mm(lhsT=wqkvT[d,f], rhs=xT[d,t])            accumulated over d
  ssq[c,t]    = mm(lhsT=esel[:,c,:], rhs=square(qkvT_c))     (RMS factors)
  rot[d',t]   = mm(lhsT=P_rot, rhs=qn)                       (RoPE pair swap)
  scoresT[s,t]= mm(lhsT=kT[:,s-blk], rhs=qT_h)               per 128-s block
  pT          = exp(scoresT)          (no max subtraction: |score|<=sqrt(128))
  attnT[d,t]  = mm(lhsT=v[s-blk,d], rhs=pT)                  accumulated over s
  denom[h,t]  = mm(lhsT=esel[:,h,:4], rhs=pT)                accumulated
  out[t,o]    = mm(lhsT=attnT_n[f,t-blk], rhs=woT[f,o])      accumulated over f
"""

import sys

sys.path.insert(0, "/opt/trn_rl_repo")

import numpy as np

import concourse.bass as bass
import concourse.tile as tile
from concourse import bacc, mybir
from concourse import bass_utils

F32 = mybir.dt.float32
F16 = mybir.dt.float16
AF = mybir.ActivationFunctionType
OP = mybir.AluOpType

DIM = 2048
N_HEADS = 16
N_KV = 4
HEAD_DIM = 128
B = 2
S = 2048
EPS = float(np.finfo(np.float32).eps)
GQ = N_HEADS // N_KV          # q heads per group = 4
GF = GQ * HEAD_DIM            # group q features = 512
P = 128
KC = DIM // P                 # 16 contraction chunks for projections
TC = 2                        # token chunks of 1024
SC = S // P                   # 16 key chunks of 128
NF = GF + 2 * HEAD_DIM        # 768 qkv features per group
FC = NF // P                  # 6 feature chunks

_CACHED_NC = None


def build_nc():
    """Build the single-core Bass program (same program for all 8 cores)."""
    nc = bacc.Bacc("TRN2", target_bir_lowering=False, debug=False,
                   num_devices=8)

    xT_d = nc.dram_tensor("xT", [TC, P, KC, 1024], F16,
                          kind="ExternalInput").ap()
    wqkvT_d = nc.dram_tensor("wqkvT", [P, KC, NF], F16,
                             kind="ExternalInput").ap()
    woT_d = nc.dram_tensor("woT", [HEAD_DIM, GQ, DIM], F16,
                           kind="ExternalInput").ap()
    cosT_d = nc.dram_tensor("cosT", [HEAD_DIM, S], F16,
                            kind="ExternalInput").ap()
    sinT_d = nc.dram_tensor("sinT", [HEAD_DIM, S], F16,
                            kind="ExternalInput").ap()
    normw_d = nc.dram_tensor("normw", [P, 2], F32, kind="ExternalInput").ap()
    prot_d = nc.dram_tensor("prot", [P, P], F16, kind="ExternalInput").ap()
    ident_d = nc.dram_tensor("ident", [P, P], F16, kind="ExternalInput").ap()
    esel_d = nc.dram_tensor("esel", [P, 5, 5], F16,
                            kind="ExternalInput").ap()
    out_d = nc.dram_tensor("out", [SC, P, GQ, 512], F16,
                           kind="ExternalOutput").ap()

    with tile.TileContext(nc) as tc:
        with (
            tc.tile_pool(name="consts", bufs=1) as cp,
        ):
            dramp = tc.alloc_tile_pool(name="dram_scratch", bufs=1,
                                       space="DRAM")
            rfac_dr = dramp.tile([5, S], F16, name="rfac_dr")
            rd_dr = [dramp.tile([4, 1024], F16, name=f"rd_dr{i}")
                     for i in range(2)]
            # stage-scoped persistent tensors (manual release for SBUF reuse)
            p1 = tc.alloc_tile_pool(name="p1", bufs=1)   # lives A (raw qkv)
            qkv_raw = p1.tile([P, 5, S], F16, name="qkv_raw")     # 20KB
            vT_sb = p1.tile([P, S], F16, name="vT_sb")            # 4KB

            p2 = tc.alloc_tile_pool(name="p2", bufs=1, side="right")  # A..D
            qk_sb = [p2.tile([P, S], F16, name=f"qk_sb{i}")
                     for i in range(5)]                           # 20KB
            v_sb = p2.tile([P, SC, HEAD_DIM], F16, name="v_sb")   # 4KB

            cos_sb = cp.tile([HEAD_DIM, S], F16, name="cos_sb")
            sin_sb = cp.tile([HEAD_DIM, S], F16, name="sin_sb")
            normw_sb = cp.tile([P, 2], F32, name="normw_sb")
            prot_sb = cp.tile([P, P], F16, name="prot_sb")
            ident_sb = cp.tile([P, P], F16, name="ident_sb")
            esel_sb = cp.tile([P, 5, 5], F16, name="esel_sb")
            eps_sb = cp.tile([P, 1], F32, name="eps_sb")
            zero_sb = cp.tile([P, 1], F32, name="zero_sb")
            nc.vector.memset(eps_sb[:], EPS)
            nc.vector.memset(zero_sb[:], 0.0)

            # ---------------- Stage A: qkv proj + rmsnorm + rope + vT ----
            # Per 1024-token chunk, k row (fc=4) first: project (PE),
            # squares (ACT) + per-row esel-matmul sums (PE) -> per-row rms
            # factor -> DRAM-broadcast, all pipelined per fc so the rope of
            # chunk t overlaps the projection of chunk t+1 with no PE gap.
            # Input DMAs for the first chunk are issued before the consts.
            with (
                tc.tile_pool(name="stA", bufs=2) as sa,
                tc.tile_pool(name="stR", bufs=3) as sr,
                tc.tile_pool(name="wq_pool", bufs=1) as wp,
                tc.tile_pool(name="psA", bufs=2, space="PSUM") as psA,
                tc.tile_pool(name="psSq", bufs=1, space="PSUM") as psSq,
                tc.tile_pool(name="psRT", bufs=2, space="PSUM") as psRT,
            ):
                wq_sb = wp.tile([P, KC, NF], F16, name="wq_sb")   # 24KB

                def emit_rope_fc(tcc, fc):
                    """RMSNorm + RoPE for one feature row of one chunk."""
                    tbase = tcc * 1024
                    rb = sr.tile([P, 1024], F16, name="rb")
                    nc.sync.dma_start(
                        rb[:], rfac_dr[fc:fc + 1, tbase:tbase + 1024]
                        .to_broadcast((P, 1024)))
                    qn = sr.tile([P, 1024], F16, name="qn")
                    wcol = 0 if fc < 4 else 1
                    nc.vector.scalar_tensor_tensor(
                        qn[:], qkv_raw[:, fc, tbase:tbase + 1024],
                        normw_sb[:, wcol:wcol + 1], rb[:],
                        op0=OP.mult, op1=OP.mult)
                    qc = sr.tile([P, 1024], F16, name="qc")
                    nc.gpsimd.tensor_mul(qc[:], qn[:],
                                         cos_sb[:, tbase:tbase + 1024])
                    for hf in range(2):
                        tsl = slice(tbase + hf * 512, tbase + hf * 512 + 512)
                        lsl = slice(hf * 512, hf * 512 + 512)
                        rot_ps = psRT.tile([P, 512], F32, name="rot_ps",
                                           tag="rt")
                        nc.tensor.matmul(rot_ps[:], prot_sb[:], qn[:, lsl],
                                         start=True, stop=True)
                        rs = sr.tile([P, 512], F16, name="rs")
                        nc.vector.tensor_mul(rs[:], rot_ps[:], sin_sb[:, tsl])
                        nc.vector.tensor_add(qk_sb[fc][:, tsl],
                                             qc[:, lsl], rs[:])

                def emit_vt(tcc):
                    """Transpose this chunk's v into [keys, vdim] blocks."""
                    for scc in range(tcc * 8, tcc * 8 + 8):
                        vt_ps = psRT.tile([P, P], F16, name="vt_ps",
                                          tag="rt")
                        nc.tensor.transpose(
                            vt_ps[:], vT_sb[:, scc * P:(scc + 1) * P],
                            ident_sb[:])
                        nc.vector.tensor_copy(v_sb[:, scc, :], vt_ps[:])

                # const DMAs, interleaved with the first chunk's loads so
                # the first matmul starts as early as possible
                const_dmas = [
                    (esel_sb, esel_d), (normw_sb, normw_d),
                    (prot_sb, prot_d), (cos_sb, cosT_d),
                    (sin_sb, sinT_d), (ident_sb, ident_d),
                ]
                FCO = (4, 0, 1, 2, 3, 5)  # k first, v last
                for tcc in range(TC):
                    tbase = tcc * 1024
                    xt = sa.tile([P, KC, 1024], F16, name="xt")   # 32KB x2
                    for i, kc2 in enumerate(range(0, KC, 2)):
                        nc.sync.dma_start(xt[:, kc2:kc2 + 2, :],
                                          xT_d[tcc, :, kc2:kc2 + 2, :])
                        if tcc == 0:
                            nc.scalar.dma_start(wq_sb[:, kc2:kc2 + 2, :],
                                                wqkvT_d[:, kc2:kc2 + 2, :])
                            if const_dmas:
                                dst, src = const_dmas.pop(0)
                                nc.gpsimd.dma_start(dst[:], src)
                    ssq_ps = psSq.tile([5, 1024], F32, name="ssq_ps")
                    for fi, fc in enumerate(FCO):
                        ps = psA.tile([P, 1024], F32, name="qkv_ps")
                        for kc in range(KC):
                            for hf in range(2):
                                hsl = slice(hf * 512, hf * 512 + 512)
                                nc.tensor.matmul(
                                    ps[:, hsl],
                                    wq_sb[:, kc, fc * P:(fc + 1) * P],
                                    xt[:, kc, hsl],
                                    start=(kc == 0), stop=(kc == KC - 1))
                        if fc != 5:
                            sq = sa.tile([P, 1024], F16, name="sq")
                            nc.scalar.activation(sq[:], ps[:], AF.Square,
                                                 bias=zero_sb[:])
                            for hf in range(2):
                                hsl = slice(hf * 512, hf * 512 + 512)
                                nc.tensor.matmul(
                                    ssq_ps[:, hsl],
                                    esel_sb[:, fc, :],
                                    sq[:, hsl],
                                    start=(fc == 4), stop=(fc == 3),
                                    skip_group_check=True)
                            nc.vector.tensor_copy(
                                qkv_raw[:, fc, tbase:tbase + 1024], ps[:])
                            # per-row rms factor -> DRAM (for broadcast).
                            # All 5 rows are processed each time (same cost,
                            # free-dim bound; engines need base partition 0)
                            # but only the just-completed row fc is shipped.
                            std = sa.tile([5, 1024], F32, name="std")
                            nc.scalar.activation(std[:], ssq_ps[:],
                                                 AF.Sqrt,
                                                 scale=1.0 / HEAD_DIM,
                                                 bias=eps_sb[0:5, :])
                            rfacf = sa.tile([5, 1024], F32, name="rfacf")
                            nc.vector.reciprocal_approx_fast(rfacf[:], std[:])
                            rfac16 = sa.tile([5, 1024], F16, name="rfac16")
                            nc.vector.tensor_copy(rfac16[:], rfacf[:])
                            nc.gpsimd.dma_start(
                                rfac_dr[fc:fc + 1, tbase:tbase + 1024],
                                rfac16[fc:fc + 1, :])
                        else:
                            nc.vector.tensor_copy(
                                vT_sb[:, tbase:tbase + 1024], ps[:])
                        # overlap previous chunk's rope under this chunk
                        if tcc == 1 and fi < 5:
                            emit_rope_fc(0, FCO[fi])
                        if tcc == 1 and fi == 5:
                            emit_vt(0)
                for fc in (4, 0, 1, 2, 3):
                    emit_rope_fc(TC - 1, fc)
                emit_vt(TC - 1)

            p1.release()

            # ---------------- Stage D: attention ------------------------
            p3 = tc.alloc_tile_pool(name="p3", bufs=1)   # lives D..E
            atn_n = [p3.tile([P, GQ, 1024], F16, name=f"atn_n{i}")
                     for i in range(2)]                           # 16KB
            woT_sb = p3.tile([P, GQ, DIM], F16, name="woT_sb")    # 16KB
            nc.sync.dma_start(woT_sb[:], woT_d)
            with (
                tc.tile_pool(name="stD", bufs=2) as sd,
                tc.tile_pool(name="atr", bufs=4) as atr,
                tc.tile_pool(name="accp", bufs=2) as accp,
                tc.tile_pool(name="ptp", bufs=4) as ptp,
                tc.tile_pool(name="psS", bufs=2, space="PSUM") as psS,
                tc.tile_pool(name="psPV", bufs=1, space="PSUM") as psPV,
                tc.tile_pool(name="psDN", bufs=1, space="PSUM") as psDN,
            ):
                def emit_pv(pv_ps, pt, scc):
                    for hf in range(2):
                        hs = slice(hf * 512, hf * 512 + 512)
                        nc.tensor.matmul(
                            pv_ps[:, hs], v_sb[:, scc, :], pt[:, hs],
                            start=(scc == 0), stop=(scc == SC - 1))

                atrs = []
                for pair in range(2):
                    po = pair * 1024
                    dn_ps = psDN.tile([4, 1024], F32, name="dn_ps")
                    for h in range(GQ):
                        pv_ps = psPV.tile([P, 1024], F32, name="pv_ps")
                        acc = accp.tile([P, 1024], F16, name="acc")
                        prev = None
                        for scc in range(SC):
                            ksl = qk_sb[4][:, scc * P:(scc + 1) * P]
                            sp = psS.tile([P, 1024], F32, name="sp")
                            for hf in range(2):
                                hs = slice(hf * 512, hf * 512 + 512)
                                nc.tensor.matmul(
                                    sp[:, hs], ksl,
                                    qk_sb[h][:, po + hf * 512:
                                              po + hf * 512 + 512],
                                    start=True, stop=True)
                            pt = ptp.tile([P, 1024], F16, name="pt")
                            nc.scalar.activation(pt[:], sp[:], AF.Exp,
                                                 bias=zero_sb[:])
                            # denominator partial sums on the (idle) DVE
                            if scc == 0:
                                nc.vector.tensor_copy(acc[:], pt[:])
                            else:
                                nc.vector.tensor_add(acc[:], acc[:], pt[:])
                            if prev is not None:
                                emit_pv(pv_ps, *prev)
                            prev = (pt, scc)
                        emit_pv(pv_ps, *prev)
                        # denominator: reduce acc over keys into row h
                        for hf in range(2):
                            hs = slice(hf * 512, hf * 512 + 512)
                            nc.tensor.matmul(
                                dn_ps[:, hs], esel_sb[:, h, 0:4],
                                acc[:, hs],
                                start=(h == 0), stop=(h == GQ - 1),
                                skip_group_check=True)
                        atn_raw = atr.tile([P, 1024], F16, name="atn_raw")
                        nc.vector.tensor_copy(atn_raw[:], pv_ps[:])
                        atrs.append((pair, h, atn_raw))
                    # softmax denominators -> reciprocal -> broadcast
                    rdf = sd.tile([4, 1024], F32, name="rdf")
                    nc.vector.reciprocal_approx_fast(rdf[:], dn_ps[:])
                    rd16 = sd.tile([4, 1024], F16, name="rd16")
                    nc.vector.tensor_copy(rd16[:], rdf[:])
                    nc.gpsimd.dma_start(rd_dr[pair][:], rd16[:])
                    for h in range(GQ):
                        rbh = sd.tile([P, 1024], F16, name="rbh")
                        nc.sync.dma_start(
                            rbh[:],
                            rd_dr[pair][h:h + 1, :].to_broadcast((P, 1024)))
                        pr, hh, araw = atrs.pop(0)
                        nc.vector.tensor_mul(atn_n[pr][:, hh, :],
                                             araw[:], rbh[:])

            p2.release()

            # ---------------- Stage E: output projection -----------------
            with (
                tc.tile_pool(name="stE", bufs=4) as se,
                tc.tile_pool(name="psE", bufs=8, space="PSUM") as psE,
            ):
                for tcc in range(SC):
                    pr = tcc // 8
                    tloc = (tcc % 8) * P
                    ps4 = [psE.tile([P, 512], F32, name="out_ps")
                           for _ in range(4)]
                    for h in range(GQ):
                        lhs = atn_n[pr][:, h, tloc:tloc + P]
                        for oc in range(4):
                            nc.tensor.matmul(
                                ps4[oc][:], lhs,
                                woT_sb[:, h, oc * 512:(oc + 1) * 512],
                                start=(h == 0), stop=(h == GQ - 1),
                                skip_group_check=True)
                    ob = se.tile([P, GQ, 512], F16, name="ob")
                    for oc in range(4):
                        if oc % 2 == 0:
                            nc.vector.tensor_copy(ob[:, oc, :], ps4[oc][:])
                        else:
                            nc.scalar.activation(ob[:, oc, :], ps4[oc][:],
                                                 AF.Copy)
                    nc.sync.dma_start(out_d[tcc], ob[:])
            p3.release()

    nc.compile()
    return nc


def make_in_maps(x, wqkv, wo, q_norm_w, k_norm_w, freqs_cos, freqs_sin):
    """Build the 8 per-core input maps. Core c = b*4 + g."""
    x = np.asarray(x, np.float32)
    wqkv = np.asarray(wqkv, np.float32)
    wo = np.asarray(wo, np.float32)
    q_norm_w = np.asarray(q_norm_w, np.float32)
    k_norm_w = np.asarray(k_norm_w, np.float32)
    cosT = np.ascontiguousarray(
        np.asarray(freqs_cos, np.float32)[:, 0, :].T).astype(np.float16)
    sinT = np.ascontiguousarray(
        np.asarray(freqs_sin, np.float32)[:, 0, :].T).astype(np.float16)

    normw = np.empty((P, 2), np.float32)
    normw[:, 0] = q_norm_w * np.float32(1.0 / np.sqrt(HEAD_DIM))
    normw[:, 1] = k_norm_w

    prot = np.zeros((P, P), np.float16)
    prot[np.arange(1, P, 2), np.arange(0, P, 2)] = -1.0
    prot[np.arange(0, P, 2), np.arange(1, P, 2)] = 1.0
    ident = np.eye(P, dtype=np.float16)
    esel = np.zeros((P, 5, 5), np.float16)
    for c in range(5):
        esel[:, c, c] = 1.0

    q_size = N_HEADS * HEAD_DIM
    kv_size = N_KV * HEAD_DIM
    in_maps = []
    for b in range(B):
        # [tc, p, kc, u]: xT[kc*128+p, tc*1024+u] pre-tiled, 2KB runs
        xT = np.ascontiguousarray(
            x[b].reshape(TC, 1024, KC, P).transpose(0, 3, 2, 1)
        ).astype(np.float16)
        for g in range(N_KV):
            wq = wqkv[g * GF:(g + 1) * GF]
            wk = wqkv[q_size + g * HEAD_DIM:q_size + (g + 1) * HEAD_DIM]
            wv = wqkv[q_size + kv_size + g * HEAD_DIM:
                      q_size + kv_size + (g + 1) * HEAD_DIM]
            wqkvT = np.ascontiguousarray(
                np.concatenate([wq, wk, wv], axis=0).T
                .reshape(KC, P, NF).transpose(1, 0, 2)).astype(np.float16)
            woT = np.ascontiguousarray(
                wo[:, g * GF:(g + 1) * GF].T.reshape(GQ, HEAD_DIM, DIM)
                .transpose(1, 0, 2)).astype(np.float16)
            in_maps.append({
                "xT": xT, "wqkvT": wqkvT, "woT": woT,
                "cosT": cosT, "sinT": sinT, "normw": normw,
                "prot": prot, "ident": ident, "esel": esel,
            })
    return in_maps


def run(in_maps, trace=False):
    global _CACHED_NC
    if _CACHED_NC is None:
        _CACHED_NC = build_nc()
    return bass_utils.run_bass_kernel_spmd(
        _CACHED_NC, in_maps, core_ids=list(range(8)), trace=trace)


def kernel(x, wqkv, wo, q_norm_w, k_norm_w, freqs_cos, freqs_sin):
    in_maps = make_in_maps(x, wqkv, wo, q_norm_w, k_norm_w,
                           freqs_cos, freqs_sin)
    res = run(in_maps, trace=False)
    out = np.zeros((B, S, DIM), np.float32)
    for b in range(B):
        for g in range(N_KV):
            o = res.results[b * N_KV + g]["out"]    # [SC, P, GQ, 512]
            out[b] += o.reshape(S, DIM).astype(np.float32)
    return out


# revision 19
# speedup vs baseline: 1.5210x; 1.0177x over previous
"""Trainium2 Bass kernel for nn_Attention_3710851743764.

Full attention block: qkv proj -> per-head RMSNorm(q,k) -> RoPE -> GQA
attention (16 q heads, 4 kv heads, S=2048, D=128) -> out proj.

Sharding: 8 cores = 2 (batch) x 4 (kv-head groups). Each core computes its
batch's qkv for its group (4 q heads + 1 kv head), full attention for those
heads, and a partial output projection (its 512 wo columns); the host sums
the 4 partials per batch.

All matmul inputs are fp16 (FWL weight loads, 2x DVE rate, half DMA);
PSUM accumulation is fp32. Softmax denominators are accumulated on the
(otherwise idle) DVE in fp16, reduced over key lanes by one tiny PE matmul
per head into PSUM row 0, and normalization is pipelined per head. The PE
is kept continuously busy (HAM stays at max p-state): RMSNorm/RoPE fuse
into the projection stage per token chunk, the second chunk's q-row RoPE
defers into the (scalar-bound) attention stage, and stage E draws its PSUM
from the attention pools' tags so no pool-teardown barrier separates them.

Dataflow is fully "transposed" (features on partitions, tokens on free):
  qkvT[f,t]   = mm(lhsT=wqkvT[d,f], rhs=xT[d,t])            accumulated over d
  ssq[c,t]    = mm(lhsT=esel[:,c,:], rhs=square(qkvT_c))     (RMS factors)
  rot[d',t]   = mm(lhsT=P_rot, rhs=qn)                       (RoPE pair swap)
  scoresT[s,t]= mm(lhsT=kT[:,s-blk], rhs=qT_h)               per 128-s block
  pT          = exp(scoresT)          (no max subtraction: |score|<=sqrt(128))
  attnT[d,t]  = mm(lhsT=v[s-blk,d], rhs=pT)                  accumulated over s
  acc[l,t]    = sum_s pT  (DVE);  denom[t] = mm(lhsT=ones, rhs=acc)
  out[t,o]    = mm(lhsT=attnT_n[f,t-blk], rhs=woT[f,o])      accumulated over f
"""

import sys

sys.path.insert(0, "/opt/trn_rl_repo")

import numpy as np

import concourse.bass as bass
import concourse.tile as tile
from concourse import bacc, mybir
from concourse import bass_utils

F32 = mybir.dt.float32
F16 = mybir.dt.float16
AF = mybir.ActivationFunctionType
OP = mybir.AluOpType

DIM = 2048
N_HEADS = 16
N_KV = 4
HEAD_DIM = 128
B = 2
S = 2048
EPS = float(np.finfo(np.float32).eps)
GQ = N_HEADS // N_KV          # q heads per group = 4
GF = GQ * HEAD_DIM            # group q features = 512
P = 128
KC = DIM // P                 # 16 contraction chunks for projections
TC = 2                        # token chunks of 1024
SC = S // P                   # 16 key chunks of 128
NF = GF + 2 * HEAD_DIM        # 768 qkv features per group
FC = NF // P                  # 6 feature chunks
FCO = (4, 0, 1, 2, 3, 5)      # k first, v last

_CACHED_NC = None


def build_nc():
    """Build the single-core Bass program (same program for all 8 cores)."""
    nc = bacc.Bacc("TRN2", target_bir_lowering=False, debug=False,
                   num_devices=8)

    xT_d = nc.dram_tensor("xT", [TC, P, KC, 1024], F16,
                          kind="ExternalInput").ap()
    wqkvT_d = nc.dram_tensor("wqkvT", [P, FC, KC, HEAD_DIM], F16,
                             kind="ExternalInput").ap()
    woT_d = nc.dram_tensor("woT", [HEAD_DIM, GQ, DIM], F16,
                           kind="ExternalInput").ap()
    cosT_d = nc.dram_tensor("cosT", [HEAD_DIM, S], F16,
                            kind="ExternalInput").ap()
    sinT_d = nc.dram_tensor("sinT", [HEAD_DIM, S], F16,
                            kind="ExternalInput").ap()
    normw_d = nc.dram_tensor("normw", [P, 2], F32, kind="ExternalInput").ap()
    prot_d = nc.dram_tensor("prot", [P, P], F16, kind="ExternalInput").ap()
    ident_d = nc.dram_tensor("ident", [P, P], F16, kind="ExternalInput").ap()
    esel_d = nc.dram_tensor("esel", [P, 5, 5], F16,
                            kind="ExternalInput").ap()
    out_d = nc.dram_tensor("out", [SC, P, GQ, 512], F16,
                           kind="ExternalOutput").ap()

    with tile.TileContext(nc) as tc:
        with (
            tc.tile_pool(name="consts", bufs=1) as cp,
        ):
            dramp = tc.alloc_tile_pool(name="dram_scratch", bufs=1,
                                       space="DRAM")
            rfac_dr = dramp.tile([5, S], F16, name="rfac_dr")
            rd_dr = [dramp.tile([4, 1024], F16, name=f"rd_dr{i}")
                     for i in range(2)]
            # stage-scoped persistent tensors (manual release for SBUF reuse)
            p1 = tc.alloc_tile_pool(name="p1", bufs=1)   # lives A..rope end
            qkv_raw = p1.tile([P, 5, S], F16, name="qkv_raw")     # 20KB
            vT_sb = p1.tile([P, S], F16, name="vT_sb")            # 4KB

            p2 = tc.alloc_tile_pool(name="p2", bufs=1, side="right")  # A..D
            qk_sb = [p2.tile([P, S], F16, name=f"qk_sb{i}")
                     for i in range(5)]                           # 20KB
            v_sb = p2.tile([P, SC, HEAD_DIM], F16, name="v_sb")   # 4KB

            cos_sb = cp.tile([HEAD_DIM, S], F16, name="cos_sb")
            sin_sb = cp.tile([HEAD_DIM, S], F16, name="sin_sb")
            normw_sb = cp.tile([P, 2], F32, name="normw_sb")
            prot_sb = cp.tile([P, P], F16, name="prot_sb")
            ident_sb = cp.tile([P, P], F16, name="ident_sb")
            esel_sb = cp.tile([P, 5, 5], F16, name="esel_sb")
            eps_sb = cp.tile([P, 1], F32, name="eps_sb")
            zero_sb = cp.tile([P, 1], F32, name="zero_sb")
            nc.vector.memset(eps_sb[:], EPS)
            nc.vector.memset(zero_sb[:], 0.0)

            # ---------------- Stage A: qkv proj + rmsnorm + rope + vT ----
            sr = tc.alloc_tile_pool(name="stR", bufs=3)  # lives into D
            sa = tc.alloc_tile_pool(name="stA", bufs=2)
            wp = tc.alloc_tile_pool(name="wq_pool", bufs=1)
            psA = tc.alloc_tile_pool(name="psA", bufs=2, space="PSUM")
            psSq = tc.alloc_tile_pool(name="psSq", bufs=1, space="PSUM")
            psRT = tc.alloc_tile_pool(name="psRT", bufs=2, space="PSUM")

            wq_sb = wp.tile([P, FC, KC, HEAD_DIM], F16, name="wq_sb")

            def emit_rope_fc(tcc, fc, rot_pool):
                """RMSNorm + RoPE for one feature row of one chunk."""
                tbase = tcc * 1024
                rb = sr.tile([P, 1024], F16, name="rb")
                nc.sync.dma_start(
                    rb[:], rfac_dr[fc:fc + 1, tbase:tbase + 1024]
                    .to_broadcast((P, 1024)))
                qn = sr.tile([P, 1024], F16, name="qn")
                wcol = 0 if fc < 4 else 1
                nc.vector.scalar_tensor_tensor(
                    qn[:], qkv_raw[:, fc, tbase:tbase + 1024],
                    normw_sb[:, wcol:wcol + 1], rb[:],
                    op0=OP.mult, op1=OP.mult)
                qc = sr.tile([P, 1024], F16, name="qc")
                nc.gpsimd.tensor_mul(qc[:], qn[:],
                                     cos_sb[:, tbase:tbase + 1024])
                for hf in range(2):
                    tsl = slice(tbase + hf * 512, tbase + hf * 512 + 512)
                    lsl = slice(hf * 512, hf * 512 + 512)
                    rot_ps = rot_pool.tile(
                        [P, 512], F32, name="rot_ps",
                        tag="rt" if rot_pool is psRT else "sp")
                    nc.tensor.matmul(rot_ps[:], prot_sb[:], qn[:, lsl],
                                     start=True, stop=True)
                    rs = sr.tile([P, 512], F16, name="rs")
                    nc.vector.tensor_mul(rs[:], rot_ps[:], sin_sb[:, tsl])
                    nc.vector.tensor_add(qk_sb[fc][:, tsl],
                                         qc[:, lsl], rs[:])

            def emit_vt(tcc):
                """Transpose this chunk's v into [keys, vdim] blocks."""
                for scc in range(tcc * 8, tcc * 8 + 8):
                    vt_ps = psRT.tile([P, P], F16, name="vt_ps", tag="rt")
                    nc.tensor.transpose(
                        vt_ps[:], vT_sb[:, scc * P:(scc + 1) * P],
                        ident_sb[:])
                    nc.vector.tensor_copy(v_sb[:, scc, :], vt_ps[:])

            const_dmas = [
                (esel_sb, esel_d), (normw_sb, normw_d),
                (prot_sb, prot_d), (cos_sb, cosT_d),
                (sin_sb, sinT_d), (ident_sb, ident_d),
            ]
            for tcc in range(TC):
                tbase = tcc * 1024
                xt = sa.tile([P, KC, 1024], F16, name="xt")   # 32KB x2
                for i, kc2 in enumerate(range(0, KC, 2)):
                    q = nc.sync if i % 2 == 0 else nc.gpsimd
                    q.dma_start(xt[:, kc2:kc2 + 2, :],
                                xT_d[tcc, :, kc2:kc2 + 2, :])
                    if tcc == 0:
                        if i < FC:
                            f = FCO[i]
                            nc.scalar.dma_start(wq_sb[:, f], wqkvT_d[:, f])
                        if i >= 1 and const_dmas:
                            dst, src = const_dmas.pop(0)
                            nc.scalar.dma_start(dst[:], src)
                ssq_ps = psSq.tile([5, 1024], F32, name="ssq_ps")
                for fi, fc in enumerate(FCO):
                    ps = psA.tile([P, 1024], F32, name="qkv_ps")
                    for kc in range(KC):
                        for hf in range(2):
                            hsl = slice(hf * 512, hf * 512 + 512)
                            nc.tensor.matmul(
                                ps[:, hsl],
                                wq_sb[:, fc, kc, :],
                                xt[:, kc, hsl],
                                start=(kc == 0), stop=(kc == KC - 1))
                    if fc != 5:
                        sq = sa.tile([P, 1024], F16, name="sq")
                        nc.scalar.activation(sq[:], ps[:], AF.Square,
                                             bias=zero_sb[:])
                        for hf in range(2):
                            hsl = slice(hf * 512, hf * 512 + 512)
                            nc.tensor.matmul(
                                ssq_ps[:, hsl],
                                esel_sb[:, fc, :],
                                sq[:, hsl],
                                start=(fc == 4), stop=(fc == 3),
                                skip_group_check=True)
                        nc.vector.tensor_copy(
                            qkv_raw[:, fc, tbase:tbase + 1024], ps[:])
                        # per-row rms factor -> DRAM (for broadcast).
                        # All 5 rows are processed each time (same cost,
                        # free-dim bound; engines need base partition 0)
                        # but only the just-completed row fc is shipped.
                        std = sa.tile([5, 1024], F32, name="std")
                        nc.scalar.activation(std[:], ssq_ps[:], AF.Sqrt,
                                             scale=1.0 / HEAD_DIM,
                                             bias=eps_sb[0:5, :])
                        rfacf = sa.tile([5, 1024], F32, name="rfacf")
                        nc.vector.reciprocal_approx_fast(rfacf[:], std[:])
                        rfac16 = sa.tile([5, 1024], F16, name="rfac16")
                        nc.vector.tensor_copy(rfac16[:], rfacf[:])
                        nc.gpsimd.dma_start(
                            rfac_dr[fc:fc + 1, tbase:tbase + 1024],
                            rfac16[fc:fc + 1, :])
                    else:
                        nc.vector.tensor_copy(
                            vT_sb[:, tbase:tbase + 1024], ps[:])
                    # overlap previous chunk's rope under this chunk
                    if tcc == 1 and fi < 5:
                        emit_rope_fc(0, FCO[fi], psRT)
                    if tcc == 1 and fi == 5:
                        emit_vt(0)
            # chunk 1: k row + v transposes now; q rows defer into stage D
            emit_rope_fc(TC - 1, 4, psRT)
            emit_vt(TC - 1)

            psRT.release()
            psSq.release()
            psA.release()
            wp.release()
            sa.release()

            # ---------------- Stage D: attention + Stage E: out proj -----
            p3 = tc.alloc_tile_pool(name="p3", bufs=1)
            atn_n = [p3.tile([P, GQ, 1024], F16, name=f"atn_n{i}")
                     for i in range(2)]                           # 16KB
            woT_sb = p3.tile([P, GQ, DIM], F16, name="woT_sb")    # 16KB
            nc.scalar.dma_start(woT_sb[:], woT_d)
            with (
                tc.tile_pool(name="stD", bufs=2) as sd,
                tc.tile_pool(name="atr", bufs=2) as atr,
                tc.tile_pool(name="accp", bufs=2) as accp,
                tc.tile_pool(name="ptp", bufs=4) as ptp,
                tc.tile_pool(name="psS", bufs=3, space="PSUM") as psS,
                tc.tile_pool(name="psPV", bufs=1, space="PSUM") as psPV,
            ):
                def emit_pv(pv_ps, pt, scc):
                    for hf in range(2):
                        hs = slice(hf * 512, hf * 512 + 512)
                        nc.tensor.matmul(
                            pv_ps[:, hs], v_sb[:, scc, :], pt[:, hs],
                            start=(scc == 0), stop=(scc == SC - 1))

                rope_defer = [(TC - 1, fc) for fc in (0, 1, 2, 3)]
                for pair in range(2):
                    po = pair * 1024
                    for h in range(GQ):
                        pv_ps = psPV.tile([P, 1024], F32, name="pv_ps")
                        acc = accp.tile([P, 1024], F16, name="acc")
                        prev = None
                        for scc in range(SC):
                            # deferred chunk-1 q-row rope, spread over the
                            # first (scalar-bound) head's iterations
                            if pair == 0 and h == 0 and scc % 4 == 0 \
                                    and rope_defer:
                                emit_rope_fc(*rope_defer.pop(0), psS)
                            ksl = qk_sb[4][:, scc * P:(scc + 1) * P]
                            sp = psS.tile([P, 1024], F32, name="sp",
                                          tag="sp")
                            for hf in range(2):
                                hs = slice(hf * 512, hf * 512 + 512)
                                nc.tensor.matmul(
                                    sp[:, hs], ksl,
                                    qk_sb[h][:, po + hf * 512:
                                              po + hf * 512 + 512],
                                    start=True, stop=True)
                            pt = ptp.tile([P, 1024], F16, name="pt")
                            nc.scalar.activation(pt[:], sp[:], AF.Exp,
                                                 bias=zero_sb[:])
                            # denominator partial sums on the (idle) DVE
                            if scc == 0:
                                nc.vector.tensor_copy(acc[:], pt[:])
                            else:
                                nc.vector.tensor_add(acc[:], acc[:], pt[:])
                            if prev is not None:
                                emit_pv(pv_ps, *prev)
                            prev = (pt, scc)
                        emit_pv(pv_ps, *prev)
                        # denominator: reduce acc over key lanes -> row 0,
                        # then reciprocal -> broadcast -> normalize, all
                        # pipelined per head
                        dnt = psS.tile([1, 1024], F32, name="dnt", tag="sp")
                        for hf in range(2):
                            hs = slice(hf * 512, hf * 512 + 512)
                            nc.tensor.matmul(
                                dnt[0:1, hs], esel_sb[:, 0, 0:1],
                                acc[:, hs], start=True, stop=True)
                        rdf = sd.tile([1, 1024], F32, name="rdf")
                        nc.vector.reciprocal_approx_fast(rdf[:], dnt[0:1, :])
                        rd16 = sd.tile([1, 1024], F16, name="rd16")
                        nc.vector.tensor_copy(rd16[:], rdf[:])
                        nc.gpsimd.dma_start(rd_dr[pair][h:h + 1, :], rd16[:])
                        rbh = sd.tile([P, 1024], F16, name="rbh")
                        nc.sync.dma_start(
                            rbh[:],
                            rd_dr[pair][h:h + 1, :].to_broadcast((P, 1024)))
                        atn_raw = atr.tile([P, 1024], F16, name="atn_raw")
                        nc.vector.tensor_copy(atn_raw[:], pv_ps[:])
                        nc.vector.tensor_mul(atn_n[pair][:, h, :],
                                             atn_raw[:], rbh[:])

                # ---------- Stage E (same pools: no teardown barrier) ----
                with tc.tile_pool(name="stE", bufs=4) as se:
                    for tcc in range(SC):
                        pr = tcc // 8
                        tloc = (tcc % 8) * P
                        o2 = [psS.tile([P, 2, 512], F32, name="out_ps",
                                       tag="sp") for _ in range(2)]
                        for h in range(GQ):
                            lhs = atn_n[pr][:, h, tloc:tloc + P]
                            for oc in range(4):
                                nc.tensor.matmul(
                                    o2[oc // 2][:, oc % 2, :], lhs,
                                    woT_sb[:, h, oc * 512:(oc + 1) * 512],
                                    start=(h == 0), stop=(h == GQ - 1),
                                    skip_group_check=True)
                        ob = se.tile([P, GQ, 512], F16, name="ob")
                        for oc in range(4):
                            if oc % 2 == 0:
                                nc.vector.tensor_copy(
                                    ob[:, oc, :], o2[oc // 2][:, oc % 2, :])
                            else:
                                nc.scalar.activation(
                                    ob[:, oc, :], o2[oc // 2][:, oc % 2, :],
                                    AF.Copy)
                        nc.sync.dma_start(out_d[tcc], ob[:])
            p2.release()
            p3.release()
            sr.release()
            p1.release()

    nc.compile()
    return nc


def make_in_maps(x, wqkv, wo, q_norm_w, k_norm_w, freqs_cos, freqs_sin):
    """Build the 8 per-core input maps. Core c = b*4 + g."""
    x = np.asarray(x, np.float32)
    wqkv = np.asarray(wqkv, np.float32)
    wo = np.asarray(wo, np.float32)
    q_norm_w = np.asarray(q_norm_w, np.float32)
    k_norm_w = np.asarray(k_norm_w, np.float32)
    cosT = np.ascontiguousarray(
        np.asarray(freqs_cos, np.float32)[:, 0, :].T).astype(np.float16)
    sinT = np.ascontiguousarray(
        np.asarray(freqs_sin, np.float32)[:, 0, :].T).astype(np.float16)

    normw = np.empty((P, 2), np.float32)
    normw[:, 0] = q_norm_w * np.float32(1.0 / np.sqrt(HEAD_DIM))
    normw[:, 1] = k_norm_w

    prot = np.zeros((P, P), np.float16)
    prot[np.arange(1, P, 2), np.arange(0, P, 2)] = -1.0
    prot[np.arange(0, P, 2), np.arange(1, P, 2)] = 1.0
    ident = np.eye(P, dtype=np.float16)
    esel = np.zeros((P, 5, 5), np.float16)
    for c in range(5):
        esel[:, c, c] = 1.0

    q_size = N_HEADS * HEAD_DIM
    kv_size = N_KV * HEAD_DIM
    in_maps = []
    for b in range(B):
        # [tc, p, kc, u]: xT[kc*128+p, tc*1024+u] pre-tiled, 2KB runs
        xT = np.ascontiguousarray(
            x[b].reshape(TC, 1024, KC, P).transpose(0, 3, 2, 1)
        ).astype(np.float16)
        for g in range(N_KV):
            wq = wqkv[g * GF:(g + 1) * GF]
            wk = wqkv[q_size + g * HEAD_DIM:q_size + (g + 1) * HEAD_DIM]
            wv = wqkv[q_size + kv_size + g * HEAD_DIM:
                      q_size + kv_size + (g + 1) * HEAD_DIM]
            # fc-major: [p, f, kc, j] = W[f*128+j, kc*128+p]
            wqkvT = np.ascontiguousarray(
                np.concatenate([wq, wk, wv], axis=0).T
                .reshape(KC, P, FC, HEAD_DIM).transpose(1, 2, 0, 3)
            ).astype(np.float16)
            woT = np.ascontiguousarray(
                wo[:, g * GF:(g + 1) * GF].T.reshape(GQ, HEAD_DIM, DIM)
                .transpose(1, 0, 2)).astype(np.float16)
            in_maps.append({
                "xT": xT, "wqkvT": wqkvT, "woT": woT,
                "cosT": cosT, "sinT": sinT, "normw": normw,
                "prot": prot, "ident": ident, "esel": esel,
            })
    return in_maps


def run(in_maps, trace=False):
    global _CACHED_NC
    if _CACHED_NC is None:
        _CACHED_NC = build_nc()
    return bass_utils.run_bass_kernel_spmd(
        _CACHED_NC, in_maps, core_ids=list(range(8)), trace=trace)


def kernel(x, wqkv, wo, q_norm_w, k_norm_w, freqs_cos, freqs_sin):
    in_maps = make_in_maps(x, wqkv, wo, q_norm_w, k_norm_w,
                           freqs_cos, freqs_sin)
    res = run(in_maps, trace=False)
    out = np.zeros((B, S, DIM), np.float32)
    for b in range(B):
        for g in range(N_KV):
            o = res.results[b * N_KV + g]["out"]    # [SC, P, GQ, 512]
            out[b] += o.reshape(S, DIM).astype(np.float32)
    return out


# revision 31
# speedup vs baseline: 1.5815x; 1.0398x over previous
"""Trainium2 Bass kernel for nn_Attention_3710851743764.

Full attention block: qkv proj -> per-head RMSNorm(q,k) -> RoPE -> GQA
attention (16 q heads, 4 kv heads, S=2048, D=128) -> out proj.

Sharding: 8 cores = 2 (batch) x 4 (kv-head groups). Each core computes its
batch's qkv for its group (4 q heads + 1 kv head), full attention for those
heads, and a partial output projection (its 512 wo columns); the host sums
the 4 partials per batch.

All matmul inputs are fp16 (FWL weight loads, 2x DVE rate, half DMA);
PSUM accumulation is fp32. Softmax denominators are accumulated on the
(otherwise idle) DVE in fp16, reduced over key lanes by one tiny PE matmul
per head into PSUM row 0, and normalization is pipelined per head. The PE
is kept continuously busy (HAM stays at max p-state): RMSNorm/RoPE fuse
into the projection stage per token chunk, the second chunk's q-row RoPE
defers into the (scalar-bound) attention stage, and stage E draws its PSUM
from the attention pools' tags so no pool-teardown barrier separates them.

Dataflow is fully "transposed" (features on partitions, tokens on free):
  qkvT[f,t]   = mm(lhsT=wqkvT[d,f], rhs=xT[d,t])            accumulated over d
  ssq[c,t]    = mm(lhsT=esel[:,c,:], rhs=square(qkvT_c))     (RMS factors)
  rot[d',t]   = mm(lhsT=P_rot, rhs=qn)                       (RoPE pair swap)
  scoresT[s,t]= mm(lhsT=kT[:,s-blk], rhs=qT_h)               per 128-s block
  pT          = exp(scoresT)          (no max subtraction: |score|<=sqrt(128))
  attnT[d,t]  = mm(lhsT=v[s-blk,d], rhs=pT)                  accumulated over s
  acc[l,t]    = sum_s pT  (DVE);  denom[t] = mm(lhsT=ones, rhs=acc)
  out[t,o]    = mm(lhsT=attnT_n[f,t-blk], rhs=woT[f,o])      accumulated over f
"""

import sys

sys.path.insert(0, "/opt/trn_rl_repo")

import numpy as np

import concourse.bass as bass
import concourse.tile as tile
from concourse import bacc, mybir
from concourse import bass_utils

F32 = mybir.dt.float32
F16 = mybir.dt.float16
AF = mybir.ActivationFunctionType
OP = mybir.AluOpType

DIM = 2048
N_HEADS = 16
N_KV = 4
HEAD_DIM = 128
B = 2
S = 2048
EPS = float(np.finfo(np.float32).eps)
GQ = N_HEADS // N_KV          # q heads per group = 4
GF = GQ * HEAD_DIM            # group q features = 512
P = 128
KC = DIM // P                 # 16 contraction chunks for projections
TC = 2                        # token chunks of 1024
SC = S // P                   # 16 key chunks of 128
NF = GF + 2 * HEAD_DIM        # 768 qkv features per group
FC = NF // P                  # 6 feature chunks
FCO = (4, 0, 1, 2, 3, 5)      # k first, v last

_CACHED_NC = None


def build_nc():
    """Build the single-core Bass program (same program for all 8 cores)."""
    nc = bacc.Bacc("TRN2", target_bir_lowering=False, debug=False,
                   num_devices=8)

    # DRAM tensors are declared with flat innermost dims so the DMA APs
    # have maximal contiguous runs (4KB+) — [16,128]-shaped APs generate
    # 256B descriptors and run ~4x slower (descriptor-rate-bound).
    xT_d = nc.dram_tensor("xT", [TC, P, KC * 1024], F16,
                          kind="ExternalInput").ap()
    wqkvT_d = nc.dram_tensor("wqkvT", [P, FC, KC * HEAD_DIM], F16,
                             kind="ExternalInput").ap()
    woT_d = nc.dram_tensor("woT", [HEAD_DIM, GQ * DIM], F16,
                           kind="ExternalInput").ap()
    cosT_d = nc.dram_tensor("cosT", [HEAD_DIM, S], F16,
                            kind="ExternalInput").ap()
    sinT_d = nc.dram_tensor("sinT", [HEAD_DIM, S], F16,
                            kind="ExternalInput").ap()
    normw_d = nc.dram_tensor("normw", [P, 2], F32, kind="ExternalInput").ap()
    prot_d = nc.dram_tensor("prot", [P, P], F16, kind="ExternalInput").ap()
    ident_d = nc.dram_tensor("ident", [P, P], F16, kind="ExternalInput").ap()
    esel_d = nc.dram_tensor("esel", [P, 5, 5], F16,
                            kind="ExternalInput").ap()
    out_d = nc.dram_tensor("out", [SC, P, GQ, 512], F16,
                           kind="ExternalOutput").ap()

    with tile.TileContext(nc) as tc:
        with (
            tc.tile_pool(name="consts", bufs=1) as cp,
        ):
            dramp = tc.alloc_tile_pool(name="dram_scratch", bufs=1,
                                       space="DRAM")
            rfac_dr = dramp.tile([5, S], F16, name="rfac_dr")
            rd_dr = [dramp.tile([4, 1024], F16, name=f"rd_dr{i}")
                     for i in range(2)]
            # stage-scoped persistent tensors (manual release for SBUF reuse)
            p1 = tc.alloc_tile_pool(name="p1", bufs=1)   # lives A..rope end
            qkv_raw = p1.tile([P, 5, S], F16, name="qkv_raw")     # 20KB
            vT_sb = p1.tile([P, S], F16, name="vT_sb")            # 4KB

            p2 = tc.alloc_tile_pool(name="p2", bufs=1, side="right")  # A..D
            qk_sb = [p2.tile([P, S], F16, name=f"qk_sb{i}")
                     for i in range(5)]                           # 20KB
            v_sb = p2.tile([P, SC, HEAD_DIM], F16, name="v_sb")   # 4KB

            cos_sb = cp.tile([HEAD_DIM, S], F16, name="cos_sb")
            sin_sb = cp.tile([HEAD_DIM, S], F16, name="sin_sb")
            normw_sb = cp.tile([P, 2], F32, name="normw_sb")
            prot_sb = cp.tile([P, P], F16, name="prot_sb")
            ident_sb = cp.tile([P, P], F16, name="ident_sb")
            esel_sb = cp.tile([P, 5, 5], F16, name="esel_sb")
            eps_sb = cp.tile([P, 1], F32, name="eps_sb")
            zero_sb = cp.tile([P, 1], F32, name="zero_sb")
            nc.vector.memset(eps_sb[:], EPS)
            nc.vector.memset(zero_sb[:], 0.0)

            # ---------------- Stage A: qkv proj + rmsnorm + rope + vT ----
            sr = tc.alloc_tile_pool(name="stR", bufs=3)  # lives into D
            sa = tc.alloc_tile_pool(name="stA", bufs=2)
            wp = tc.alloc_tile_pool(name="wq_pool", bufs=1)
            psA = tc.alloc_tile_pool(name="psA", bufs=2, space="PSUM")
            psSq = tc.alloc_tile_pool(name="psSq", bufs=1, space="PSUM")
            psRT = tc.alloc_tile_pool(name="psRT", bufs=2, space="PSUM")

            wq_sb = wp.tile([P, FC, KC * HEAD_DIM], F16, name="wq_sb")

            def rope_rb(tcc, fc, name="rb", bufs=None):
                """Issue the rms-factor broadcast DMA for one row."""
                tbase = tcc * 1024
                kw = {} if bufs is None else {"bufs": bufs}
                rb = sr.tile([P, 1024], F16, name=name, **kw)
                nc.sync.dma_start(
                    rb[:], rfac_dr[fc:fc + 1, tbase:tbase + 1024]
                    .to_broadcast((P, 1024)))
                return rb

            def emit_rope_fc(tcc, fc, rot_pool, rb=None):
                """RMSNorm + RoPE for one feature row of one chunk."""
                tbase = tcc * 1024
                if rb is None:
                    rb = rope_rb(tcc, fc)
                qn = sr.tile([P, 1024], F16, name="qn")
                wcol = 0 if fc < 4 else 1
                nc.vector.scalar_tensor_tensor(
                    qn[:], qkv_raw[:, fc, tbase:tbase + 1024],
                    normw_sb[:, wcol:wcol + 1], rb[:],
                    op0=OP.mult, op1=OP.mult)
                qc = sr.tile([P, 1024], F16, name="qc")
                nc.gpsimd.tensor_mul(qc[:], qn[:],
                                     cos_sb[:, tbase:tbase + 1024])
                for hf in range(2):
                    tsl = slice(tbase + hf * 512, tbase + hf * 512 + 512)
                    lsl = slice(hf * 512, hf * 512 + 512)
                    rot_ps = rot_pool.tile(
                        [P, 512], F32, name="rot_ps",
                        tag="rt" if rot_pool is psRT else "sp")
                    nc.tensor.matmul(rot_ps[:], prot_sb[:], qn[:, lsl],
                                     start=True, stop=True)
                    rs = sr.tile([P, 512], F16, name="rs")
                    nc.vector.tensor_mul(rs[:], rot_ps[:], sin_sb[:, tsl])
                    nc.vector.tensor_add(qk_sb[fc][:, tsl],
                                         qc[:, lsl], rs[:])

            def emit_vt(tcc):
                """Transpose this chunk's v into [keys, vdim] blocks."""
                for scc in range(tcc * 8, tcc * 8 + 8):
                    vt_ps = psRT.tile([P, P], F16, name="vt_ps", tag="rt")
                    nc.tensor.transpose(
                        vt_ps[:], vT_sb[:, scc * P:(scc + 1) * P],
                        ident_sb[:])
                    nc.vector.tensor_copy(v_sb[:, scc, :], vt_ps[:])

            const_dmas = [
                (esel_sb, esel_d), (normw_sb, normw_d),
                (prot_sb, prot_d), (cos_sb, cosT_d),
                (sin_sb, sinT_d), (ident_sb, ident_d),
            ]
            # x chunks round-robin across all three DMA queues in kc order
            # (per-queue transfer throughput, not dispatch, is the early
            # bottleneck); wq f-blocks just-in-time on the scalar queue
            # (fc=4's block first — the very first matmul needs it);
            # consts trail on gpsimd.
            for tcc in range(TC):
                tbase = tcc * 1024
                xt = sa.tile([P, KC * 1024], F16, name="xt")   # 32KB x2
                if tcc == 0:
                    nc.scalar.dma_start(wq_sb[:, 4], wqkvT_d[:, 4])
                wq_left = [0, 1, 2, 3, 5]
                for i, kc2 in enumerate(range(0, KC, 2)):
                    q = (nc.sync, nc.scalar, nc.gpsimd)[i % 3]
                    csl = slice(kc2 * 1024, (kc2 + 2) * 1024)
                    q.dma_start(xt[:, csl], xT_d[tcc, :, csl])
                    if tcc == 0:
                        if i % 3 == 1 and wq_left:
                            f = wq_left.pop(0)
                            nc.scalar.dma_start(wq_sb[:, f], wqkvT_d[:, f])
                        if i % 3 == 2 and const_dmas:
                            dst, src = const_dmas.pop(0)
                            nc.gpsimd.dma_start(dst[:], src)
                if tcc == 0:
                    for f in wq_left:
                        nc.scalar.dma_start(wq_sb[:, f], wqkvT_d[:, f])
                    for dst, src in const_dmas:
                        nc.gpsimd.dma_start(dst[:], src)
                    const_dmas = []
                ssq_ps = psSq.tile([5, 1024], F32, name="ssq_ps")
                for fi, fc in enumerate(FCO):
                    ps = psA.tile([P, 1024], F32, name="qkv_ps")
                    for kc in range(KC):
                        for hf in range(2):
                            hsl = slice(hf * 512, hf * 512 + 512)
                            nc.tensor.matmul(
                                ps[:, hsl],
                                wq_sb[:, fc, kc * P:(kc + 1) * P],
                                xt[:, kc * 1024 + hf * 512:
                                   kc * 1024 + hf * 512 + 512],
                                start=(kc == 0), stop=(kc == KC - 1))
                    if fc != 5:
                        sq = sa.tile([P, 1024], F16, name="sq")
                        nc.scalar.activation(sq[:], ps[:], AF.Square,
                                             bias=zero_sb[:])
                        for hf in range(2):
                            hsl = slice(hf * 512, hf * 512 + 512)
                            nc.tensor.matmul(
                                ssq_ps[:, hsl],
                                esel_sb[:, fc, :],
                                sq[:, hsl],
                                start=(fc == 4), stop=(fc == 3),
                                skip_group_check=True)
                        nc.vector.tensor_copy(
                            qkv_raw[:, fc, tbase:tbase + 1024], ps[:])
                        # per-row rms factor -> DRAM (for broadcast).
                        # All 5 rows are processed each time (same cost,
                        # free-dim bound; engines need base partition 0)
                        # but only the just-completed row fc is shipped.
                        std = sa.tile([5, 1024], F32, name="std")
                        nc.scalar.activation(std[:], ssq_ps[:], AF.Sqrt,
                                             scale=1.0 / HEAD_DIM,
                                             bias=eps_sb[0:5, :])
                        rfacf = sa.tile([5, 1024], F32, name="rfacf")
                        nc.vector.reciprocal_approx_fast(rfacf[:], std[:])
                        rfac16 = sa.tile([5, 1024], F16, name="rfac16")
                        nc.vector.tensor_copy(rfac16[:], rfacf[:])
                        nc.gpsimd.dma_start(
                            rfac_dr[fc:fc + 1, tbase:tbase + 1024],
                            rfac16[fc:fc + 1, :])
                    else:
                        nc.vector.tensor_copy(
                            vT_sb[:, tbase:tbase + 1024], ps[:])
                    # overlap previous chunk's rope under this chunk
                    if tcc == 1 and fi < 5:
                        emit_rope_fc(0, FCO[fi], psRT)
                    if tcc == 1 and fi == 5:
                        emit_vt(0)
            # chunk 1: k row + v transposes now; q rows defer into stage D
            emit_rope_fc(TC - 1, 4, psRT)
            emit_vt(TC - 1)

            psRT.release()
            psSq.release()
            psA.release()
            wp.release()
            sa.release()

            # ---------------- Stage D: attention + Stage E: out proj -----
            p3 = tc.alloc_tile_pool(name="p3", bufs=1)
            atn_n = [p3.tile([P, GQ, 1024], F16, name=f"atn_n{i}")
                     for i in range(2)]                           # 16KB
            woT_sb = p3.tile([P, GQ * DIM], F16, name="woT_sb")   # 16KB
            nc.scalar.dma_start(woT_sb[:], woT_d)
            with (
                tc.tile_pool(name="stD", bufs=2) as sd,
                tc.tile_pool(name="atr", bufs=2) as atr,
                tc.tile_pool(name="accp", bufs=2) as accp,
                tc.tile_pool(name="ptp", bufs=4) as ptp,
                tc.tile_pool(name="psS", bufs=3, space="PSUM") as psS,
                tc.tile_pool(name="psPV", bufs=1, space="PSUM") as psPV,
            ):
                def emit_pv(pv_ps, pt, scc):
                    for hf in range(2):
                        hs = slice(hf * 512, hf * 512 + 512)
                        nc.tensor.matmul(
                            pv_ps[:, hs], v_sb[:, scc, :], pt[:, hs],
                            start=(scc == 0), stop=(scc == SC - 1))

                def emit_dn(pair, h, acc, atn_raw):
                    """Denominator reduce -> reciprocal -> broadcast ->
                    normalize for one head (emitted one head late so the
                    PE never waits on the DVE's last acc add)."""
                    dnt = psS.tile([1, 1024], F32, name="dnt", tag="sp")
                    for hf in range(2):
                        hs = slice(hf * 512, hf * 512 + 512)
                        nc.tensor.matmul(
                            dnt[0:1, hs], esel_sb[:, 0, 0:1],
                            acc[:, hs], start=True, stop=True)
                    rdf = sd.tile([1, 1024], F32, name="rdf")
                    nc.vector.reciprocal_approx_fast(rdf[:], dnt[0:1, :])
                    rd16 = sd.tile([1, 1024], F16, name="rd16")
                    nc.vector.tensor_copy(rd16[:], rdf[:])
                    nc.gpsimd.dma_start(rd_dr[pair][h:h + 1, :], rd16[:])
                    rbh = sd.tile([P, 1024], F16, name="rbh")
                    nc.sync.dma_start(
                        rbh[:],
                        rd_dr[pair][h:h + 1, :].to_broadcast((P, 1024)))
                    nc.vector.tensor_mul(atn_n[pair][:, h, :],
                                         atn_raw[:], rbh[:])

                rope_defer = [(TC - 1, fc) for fc in (0, 1, 2, 3)]
                rope_rbs = [rope_rb(t, f, name="rbd", bufs=4)
                            for t, f in rope_defer]
                pend_dn = None
                for pair in range(2):
                    po = pair * 1024
                    for h in range(GQ):
                        pv_ps = psPV.tile([P, 1024], F32, name="pv_ps")
                        acc = accp.tile([P, 1024], F16, name="acc")
                        prev = None
                        for scc in range(SC):
                            # deferred chunk-1 q-row rope, spread over the
                            # first (scalar-bound) head's iterations
                            if pair == 0 and h == 0 and scc % 4 == 2 \
                                    and rope_defer:
                                emit_rope_fc(*rope_defer.pop(0), psS,
                                             rb=rope_rbs.pop(0))
                            ksl = qk_sb[4][:, scc * P:(scc + 1) * P]
                            sp = psS.tile([P, 1024], F32, name="sp",
                                          tag="sp")
                            for hf in range(2):
                                hs = slice(hf * 512, hf * 512 + 512)
                                nc.tensor.matmul(
                                    sp[:, hs], ksl,
                                    qk_sb[h][:, po + hf * 512:
                                              po + hf * 512 + 512],
                                    start=True, stop=True)
                            pt = ptp.tile([P, 1024], F16, name="pt")
                            nc.scalar.activation(pt[:], sp[:], AF.Exp,
                                                 bias=zero_sb[:])
                            # denominator partial sums on the (idle) DVE
                            if scc == 0:
                                nc.vector.tensor_copy(acc[:], pt[:])
                            else:
                                nc.vector.tensor_add(acc[:], acc[:], pt[:])
                            if prev is not None:
                                emit_pv(pv_ps, *prev)
                            prev = (pt, scc)
                            if scc == 2 and pend_dn is not None:
                                emit_dn(*pend_dn)
                                pend_dn = None
                        emit_pv(pv_ps, *prev)
                        atn_raw = atr.tile([P, 1024], F16, name="atn_raw")
                        nc.vector.tensor_copy(atn_raw[:], pv_ps[:])
                        pend_dn = (pair, h, acc, atn_raw)
                emit_dn(*pend_dn)

                # ---------- Stage E (same pools: no teardown barrier) ----
                with tc.tile_pool(name="stE", bufs=4) as se:
                    for tcc in range(SC):
                        pr = tcc // 8
                        tloc = (tcc % 8) * P
                        o2 = [psS.tile([P, 2, 512], F32, name="out_ps",
                                       tag="sp") for _ in range(2)]
                        for h in range(GQ):
                            lhs = atn_n[pr][:, h, tloc:tloc + P]
                            for oc in range(4):
                                nc.tensor.matmul(
                                    o2[oc // 2][:, oc % 2, :], lhs,
                                    woT_sb[:, h * DIM + oc * 512:
                                           h * DIM + oc * 512 + 512],
                                    start=(h == 0), stop=(h == GQ - 1),
                                    skip_group_check=True)
                        ob = se.tile([P, GQ, 512], F16, name="ob")
                        for oc in range(4):
                            if oc % 2 == 0:
                                nc.vector.tensor_copy(
                                    ob[:, oc, :], o2[oc // 2][:, oc % 2, :])
                            else:
                                nc.scalar.activation(
                                    ob[:, oc, :], o2[oc // 2][:, oc % 2, :],
                                    AF.Copy)
                        oq = (nc.sync, nc.gpsimd, nc.scalar)[tcc % 3]
                        oq.dma_start(out_d[tcc], ob[:])
            p2.release()
            p3.release()
            sr.release()
            p1.release()

    nc.compile()
    return nc


def make_in_maps(x, wqkv, wo, q_norm_w, k_norm_w, freqs_cos, freqs_sin):
    """Build the 8 per-core input maps. Core c = b*4 + g."""
    x = np.asarray(x, np.float32)
    wqkv = np.asarray(wqkv, np.float32)
    wo = np.asarray(wo, np.float32)
    q_norm_w = np.asarray(q_norm_w, np.float32)
    k_norm_w = np.asarray(k_norm_w, np.float32)
    cosT = np.ascontiguousarray(
        np.asarray(freqs_cos, np.float32)[:, 0, :].T).astype(np.float16)
    sinT = np.ascontiguousarray(
        np.asarray(freqs_sin, np.float32)[:, 0, :].T).astype(np.float16)

    normw = np.empty((P, 2), np.float32)
    normw[:, 0] = q_norm_w * np.float32(1.0 / np.sqrt(HEAD_DIM))
    normw[:, 1] = k_norm_w

    prot = np.zeros((P, P), np.float16)
    prot[np.arange(1, P, 2), np.arange(0, P, 2)] = -1.0
    prot[np.arange(0, P, 2), np.arange(1, P, 2)] = 1.0
    ident = np.eye(P, dtype=np.float16)
    esel = np.zeros((P, 5, 5), np.float16)
    for c in range(5):
        esel[:, c, c] = 1.0

    q_size = N_HEADS * HEAD_DIM
    kv_size = N_KV * HEAD_DIM
    in_maps = []
    for b in range(B):
        # [tc, p, kc*1024+u]: xT[kc*128+p, tc*1024+u] pre-tiled, 4KB runs
        xT = np.ascontiguousarray(
            x[b].reshape(TC, 1024, KC, P).transpose(0, 3, 2, 1)
        ).astype(np.float16).reshape(TC, P, KC * 1024)
        for g in range(N_KV):
            wq = wqkv[g * GF:(g + 1) * GF]
            wk = wqkv[q_size + g * HEAD_DIM:q_size + (g + 1) * HEAD_DIM]
            wv = wqkv[q_size + kv_size + g * HEAD_DIM:
                      q_size + kv_size + (g + 1) * HEAD_DIM]
            # fc-major: [p, f, kc*128+j] = W[f*128+j, kc*128+p]
            wqkvT = np.ascontiguousarray(
                np.concatenate([wq, wk, wv], axis=0).T
                .reshape(KC, P, FC, HEAD_DIM).transpose(1, 2, 0, 3)
            ).astype(np.float16).reshape(P, FC, KC * HEAD_DIM)
            woT = np.ascontiguousarray(
                wo[:, g * GF:(g + 1) * GF].T.reshape(GQ, HEAD_DIM, DIM)
                .transpose(1, 0, 2)).astype(np.float16) \
                .reshape(HEAD_DIM, GQ * DIM)
            in_maps.append({
                "xT": xT, "wqkvT": wqkvT, "woT": woT,
                "cosT": cosT, "sinT": sinT, "normw": normw,
                "prot": prot, "ident": ident, "esel": esel,
            })
    return in_maps


def run(in_maps, trace=False):
    global _CACHED_NC
    if _CACHED_NC is None:
        _CACHED_NC = build_nc()
    return bass_utils.run_bass_kernel_spmd(
        _CACHED_NC, in_maps, core_ids=list(range(8)), trace=trace)


def kernel(x, wqkv, wo, q_norm_w, k_norm_w, freqs_cos, freqs_sin):
    in_maps = make_in_maps(x, wqkv, wo, q_norm_w, k_norm_w,
                           freqs_cos, freqs_sin)
    res = run(in_maps, trace=False)
    out = np.zeros((B, S, DIM), np.float32)
    for b in range(B):
        for g in range(N_KV):
            o = res.results[b * N_KV + g]["out"]    # [SC, P, GQ, 512]
            out[b] += o.reshape(S, DIM).astype(np.float32)
    return out
